# revision 38
# baseline (speedup 1.0000x reference)
"""Distributed Trainium2 Bass kernel for GQA attention prefill.

Problem: B=2, S=2048, D=4096, 32 q heads, 8 kv heads, head_dim=128, RoPE,
causal mask, start_pos=0.

Sharding (8 cores): DP2 over batch x TP4 over heads.  Core c = b*4 + g gets
batch b, q-heads 8g..8g+7, kv-heads 2g..2g+1, wo rows for those q-heads.
Each core computes a partial [S, D] output (bf16); the host sums the 4
partials per batch (the row-parallel wo unshard).

All layout work happens on the host: x arrives pre-transposed and pre-cast
to bf16, weights arrive bf16 pre-tiled, RoPE tables arrive precomputed in
their on-chip [128, S] layout.

On-core dataflow per half (1024 query positions):
  load xT bf16 tiles [128, 1024]; QKV projection (bf16 matmuls); RoPE on
  the projection PSUM.  Attention runs a pair-pipelined software pipeline:
  each PAIR of 128-key score tiles lands in one 2-bank PSUM pool tile, exp
  runs one ACTIVATE per pair (halving ACT instruction overhead), a DVE
  pre-reduction sums each full pair of P tiles so the softmax denominator
  needs one ones-matmul per pair instead of two (cuts PE rowsum cycles),
  and the causal mask is accumulated by an extra PE matmul (ident x mask)
  instead of a DVE add, keeping DVE off the critical path.  PE bubbles in
  the ACT/DVE-paced attention phases are filled with injected matmuls:
  the next half's K-head projections during the first half's attention,
  and the wo output projection for s<1024 during the second half's.  The
  remaining wo work runs as a tail (wo streamed once per s-half), with
  PSUM->SBUF copies on ACT and output DMAs alternating rings.
"""

import math

import numpy as np
import ml_dtypes

import concourse.bass as bass  # noqa: F401  (bass types via bacc)
import concourse.mybir as mybir
from concourse import bacc
from concourse.bass_utils import run_bass_kernel_spmd
from concourse.tile import TileContext

F32 = mybir.dt.float32
BF16 = mybir.dt.bfloat16
NPBF = ml_dtypes.bfloat16

B, S, D = 2, 2048, 4096
NH, NKV, HD = 32, 8, 128
NCORES = 8
TPG = 4                  # tensor-parallel groups
NQL = NH // TPG          # 8 local q heads
NKVL = NKV // TPG        # 2 local kv heads
SCW = 512                # s-chunk width
HW = S // 2              # half width (1024)
NKC = D // 128           # 32 contraction chunks for projections
NTC = S // 128           # 16 T-chunks (key positions)
SCALE = 1.0 / math.sqrt(HD)
NEG = -1e9
MERGE_EXP = True      # one ACTIVATE per adjacent-slot score pair
PREREDUCE = True      # DVE pair pre-reduction + single rowsum matmul
DEBUG_DUMP = False    # add debug DRAM outputs (sim bisection only)


def _build():
    nc = bacc.Bacc("TRN2", target_bir_lowering=False, debug=False,
                   num_devices=NCORES)
    # x pre-transposed + pre-cast: [D, S] bf16
    xt_d = nc.declare_dram_parameter("xt", [D, S], BF16, isOutput=False)
    # weights arrive pre-tiled bf16: [128, m-major kc-major cols]
    wq = nc.declare_dram_parameter("wq", [128, NQL * NKC * HD], BF16, isOutput=False)
    wk = nc.declare_dram_parameter("wk", [128, NKVL * NKC * HD], BF16, isOutput=False)
    wv = nc.declare_dram_parameter("wv", [128, NKVL * NKC * HD], BF16, isOutput=False)
    wo = nc.declare_dram_parameter("wo", [128, (D // SCW) * NQL * SCW], BF16, isOutput=False)
    # RoPE tables in on-chip layout [128, S]
    cosp = nc.declare_dram_parameter("cosp", [128, S], BF16, isOutput=False)
    sinp = nc.declare_dram_parameter("sinp", [128, S], BF16, isOutput=False)
    out = nc.declare_dram_parameter("out", [S, D], BF16, isOutput=True)
    if DEBUG_DUMP:
        dbg = {
            "dbg_ksb": nc.declare_dram_parameter("dbg_ksb", [128, NKVL * S], BF16, isOutput=True),
            "dbg_vsb": nc.declare_dram_parameter("dbg_vsb", [128, NTC * NKVL * HD], BF16, isOutput=True),
            "dbg_q0": nc.declare_dram_parameter("dbg_q0", [128, HW], BF16, isOutput=True),
            "dbg_at0": nc.declare_dram_parameter("dbg_at0", [128, S], BF16, isOutput=True),
            "dbg_rec": nc.declare_dram_parameter("dbg_rec", [128, 4 * SCW], F32, isOutput=True),
            "dbg_po": nc.declare_dram_parameter("dbg_po", [128, 4 * SCW], F32, isOutput=True),
        }

    NM = NQL + 2 * NKVL

    with TileContext(nc) as tc:
        with (
            tc.tile_pool(name="const", bufs=1) as const,
            tc.tile_pool(name="big", bufs=1) as big,
            tc.tile_pool(name="sb", bufs=3) as sb,
            tc.tile_pool(name="ps", bufs=1, space="PSUM") as ps,
        ):
            # ---- weight-slice loader ----
            wsls = {}

            def load_wsl(hf, m):
                if (hf, m) in wsls:
                    return wsls[(hf, m)]
                wsl = sb.tile([128, NKC * HD], BF16, name=f"w{hf}_{m}",
                              tag="wsl", bufs=4)
                if m < NQL:
                    base = wq
                    m0 = m
                elif m < NQL + NKVL:
                    base = wk
                    m0 = m - NQL
                else:
                    base = wv
                    m0 = m - NQL - NKVL
                qw = NKC * HD // 4
                for q4 in range(4):
                    c0 = m0 * NKC * HD + q4 * qw
                    # split the weight stream across two rings so the
                    # Q-head loops never catch up with a clogged DMA queue
                    eng = nc.scalar if q4 % 2 == 1 else nc.gpsimd
                    eng.dma_start(out=wsl[:, q4 * qw:(q4 + 1) * qw],
                                  in_=base[:, c0:c0 + qw])
                wsls[(hf, m)] = wsl
                return wsl

            # K/V head weights for the grouped pass: interleave the four
            # column-quarters across heads so every head's first quarter
            # lands before any head's second quarter
            for m0_ in range(NQL, NQL + 2 * NKVL):
                wsls[(0, m0_)] = sb.tile([128, NKC * HD], BF16,
                                         name=f"w0_{m0_}", tag="wsl", bufs=4)
            qw0 = NKC * HD // 4
            for q4 in range(4):
                for m0_ in range(NQL, NQL + 2 * NKVL):
                    if m0_ < NQL + NKVL:
                        base0 = wk
                        mb0 = m0_ - NQL
                    else:
                        base0 = wv
                        mb0 = m0_ - NQL - NKVL
                    c0 = mb0 * NKC * HD + q4 * qw0
                    eng0 = nc.scalar if q4 % 2 == 1 else nc.gpsimd
                    eng0.dma_start(
                        out=wsls[(0, m0_)][:, q4 * qw0:(q4 + 1) * qw0],
                        in_=base0[:, c0:c0 + qw0])

            # ---- constants ----
            ident = const.tile([128, 128], BF16, name="ident")
            nc.gpsimd.memset(ident[:], 0.0)
            nc.gpsimd.affine_select(
                out=ident[:], in_=ident[:],
                compare_op=mybir.AluOpType.not_equal, fill=1.0,
                base=0, pattern=[[-1, 128]], channel_multiplier=1,
            )
            ones = const.tile([128, 128], BF16, name="ones")
            nc.gpsimd.memset(ones[:], 1.0)
            # causal mask as a bf16 matmul operand: applied to the score
            # PSUM via an accumulating PE matmul (keeps the mask off DVE)
            maskbf = const.tile([128, 128], BF16, name="maskbf")
            nc.gpsimd.memset(maskbf[:], 0.0)
            nc.gpsimd.affine_select(
                out=maskbf[:], in_=maskbf[:],
                compare_op=mybir.AluOpType.is_ge, fill=NEG,
                base=0, pattern=[[1, 128]], channel_multiplier=-1,
            )
            cos2 = const.tile([128, S], BF16, name="cos2")
            sin2n = const.tile([128, S], BF16, name="sin2n")
            for hh in range(2):
                hsl = slice(hh * (S // 2), (hh + 1) * (S // 2))
                nc.scalar.dma_start(out=cos2[:, hsl], in_=cosp[:, hsl])
                nc.scalar.dma_start(out=sin2n[:, hsl], in_=sinp[:, hsl])

            # PE p-state warmup: dummy matmuls ramp the array to full clock
            # (touch every PSUM tag: 2 proj + 2 sc pair-tiles + 2 o)
            warm_dsts = []
            for wi in range(2):
                warm_dsts.append(ps.tile([128, SCW], F32, name=f"pwp{wi}",
                                         tag="proj", bufs=2))
            for wi in range(2):
                wsc = ps.tile([128, 2 * SCW], F32, name=f"pws{wi}",
                              tag="sc", bufs=2)
                warm_dsts.append(wsc[:, 0:SCW])
                warm_dsts.append(wsc[:, SCW:2 * SCW])
            for wi in range(2):
                warm_dsts.append(ps.tile([128, SCW], F32, name=f"pwo{wi}",
                                         tag="o", bufs=2))
            ksb = big.tile([128, NKVL * S], BF16, name="ksb")
            vsb = big.tile([128, NTC * NKVL * HD], BF16, name="vsb")
            # attention outputs for the full sequence
            attnT = [big.tile([128, S], BF16, name=f"at{h}") for h in range(NQL)]

            # warmup copies scribble into attnT (overwritten later)
            for wi, pw in enumerate(warm_dsts):
                nc.tensor.matmul(pw, cos2[:, 0:128], cos2[:, 0:SCW],
                                 start=True, stop=True)
                nc.vector.tensor_copy(
                    out=attnT[wi // 4][:, (wi % 4) * SCW:(wi % 4 + 1) * SCW],
                    in_=pw)

            # ---- xT loads: per (hf, kc) one tile [128, 1024] ----
            xts = {}

            def load_xt(hf):
                for kc in range(NKC):
                    t = sb.tile([128, HW], BF16, name=f"xt{hf}_{kc}",
                                tag="xt", bufs=32)
                    if kc % 2 == 0:
                        eng = nc.sync
                    elif hf == 0:
                        eng = nc.scalar
                    else:
                        eng = nc.gpsimd
                    eng.dma_start(
                        out=t[:],
                        in_=xt_d[kc * 128:(kc + 1) * 128, hf * HW:(hf + 1) * HW])
                    xts[(hf, kc)] = t

            load_xt(0)

            def rope_or_v(hf, m, scq, pp, qtiles):
                sc = hf * 2 + scq
                ssl = slice(sc * SCW, (sc + 1) * SCW)
                qsl = slice(scq * SCW, (scq + 1) * SCW)
                if m < NQL + NKVL:
                    if m < NQL:
                        dst = qtiles[m][:, qsl]
                    else:
                        kv = m - NQL
                        dst = ksb[:, kv * S + sc * SCW:kv * S + (sc + 1) * SCW]
                    t1 = sb.tile([128, SCW], BF16, name=f"t1_{hf}_{m}_{scq}",
                                 tag="t1", bufs=2)
                    t2 = sb.tile([128, SCW], BF16, name=f"t2_{hf}_{m}_{scq}",
                                 tag="t2", bufs=2)
                    nc.vector.tensor_tensor(
                        out=t1[0:64, :], in0=pp[64:128, :],
                        in1=sin2n[0:64, ssl], op=mybir.AluOpType.mult)
                    nc.vector.tensor_tensor(
                        out=t1[64:128, :], in0=pp[0:64, :],
                        in1=sin2n[64:128, ssl], op=mybir.AluOpType.mult)
                    nc.vector.tensor_tensor(
                        out=t2[:], in0=pp[:], in1=cos2[:, ssl],
                        op=mybir.AluOpType.mult)
                    nc.vector.tensor_tensor(
                        out=dst, in0=t1[:], in1=t2[:],
                        op=mybir.AluOpType.add)
                else:
                    kv = m - NQL - NKVL
                    vts = sb.tile([128, SCW], BF16, name=f"vts{hf}_{kv}_{scq}",
                                  tag="vts", bufs=4)
                    nc.scalar.copy(out=vts[:], in_=pp)
                    transpose_v(hf, kv, scq, vts)

            def transpose_v(hf, kv, scq, vts):
                sc = hf * 2 + scq
                for j in range(SCW // 128):
                    pv = ps.tile([128, 128], BF16,
                                 name=f"pv{hf}_{kv}_{scq}_{j}",
                                 tag="sc", bufs=2)
                    nc.tensor.transpose(
                        pv[:], vts[:, j * 128:(j + 1) * 128], ident[:])
                    slot = (sc * 4 + j) * NKVL + kv
                    nc.scalar.copy(
                        out=vsb[:, slot * HD:(slot + 1) * HD], in_=pv[:])

            # ---- wo output projection as a resumable generator ----------
            oblk = NQL * SCW
            wots = {}

            def load_wot(li, dc):
                wot = sb.tile([128, NQL * SCW], BF16, name=f"wot{li}_{dc}",
                              tag="wot", bufs=2)
                qw = oblk // 4
                for q4 in range(4):
                    nc.gpsimd.dma_start(
                        out=wot[:, q4 * qw:(q4 + 1) * qw],
                        in_=wo[:, dc * oblk + q4 * qw:dc * oblk + (q4 + 1) * qw])
                wots[(li, dc)] = wot

            # after the attention phases complete, the sc/o PSUM banks are
            # idle: drain-mode wo accumulators cycle through all three tags
            # (6-deep pipeline) so dc-boundary os-copy chains never stall PE
            wo_drain = [False]

            def wo_gen(li, ssub_lo, ssub_hi, last_pass):
                """Yields once per wo matmul; boundary ops emitted inline."""
                load_wot(li, 0)
                load_wot(li, 1)
                cnt = 0
                for dc in range(D // SCW):
                    wot = wots[(li, dc)]
                    if dc + 2 < D // SCW:
                        load_wot(li, dc + 2)
                    for ssub in range(ssub_lo, ssub_hi):
                        s0 = ssub * 128
                        tg = ("proj", "sc", "o")[cnt % 3] if wo_drain[0] else "proj"
                        pd = ps.tile([128, SCW], F32, name=f"pd{li}_{dc}_{ssub}",
                                     tag=tg, bufs=2)
                        for kc8 in range(NQL):
                            nc.tensor.matmul(
                                pd[:],
                                attnT[kc8][:, s0:s0 + 128],
                                wot[:, kc8 * SCW:(kc8 + 1) * SCW],
                                start=(kc8 == 0), stop=(kc8 == NQL - 1))
                            yield
                        os_ = sb.tile([128, SCW], BF16, name=f"os{li}_{dc}_{ssub}",
                                      tag="os", bufs=4)
                        if wo_drain[0] and cnt % 2 == 0:
                            nc.scalar.copy(out=os_[:], in_=pd[:])
                        else:
                            nc.vector.tensor_copy(out=os_[:], in_=pd[:])
                        if last_pass and dc == D // SCW - 1:
                            oeng = (nc.sync, nc.gpsimd, nc.scalar)[ssub % 3]
                        else:
                            oeng = nc.sync if ssub % 2 == 0 else nc.gpsimd
                        oeng.dma_start(
                            out=out[s0:s0 + 128, dc * SCW:(dc + 1) * SCW],
                            in_=os_[:])
                        cnt += 1

            # ---- attention for one half: pair-pipelined ------------------
            def attention_half(hf, qtiles, filler, fill_rate):
                pairs = []
                for h in range(NQL):
                    for scq in range(2):
                        sc = hf * 2 + scq
                        for k in range(2 * sc + 2):
                            pairs.append((h, scq, k))
                n = len(pairs)
                psps = {}    # (h, scq, k) -> score pair tile [128, 1024] PSUM
                pts = {}     # (h, scq, k) -> pt pair tile [128, 1024]
                tmps = {}    # (h, scq, k) -> prereduced tile (full pairs)
                pos = {}
                prs = {}
                fill_acc = [0.0]

                def geom(sc, tcx):
                    j = tcx - 4 * sc
                    off = j * 128 if j > 0 else 0
                    return j, off, SCW - off

                def stage_a(h, scq, k):
                    # scores for both tiles of the pair + diagonal masks
                    sc = hf * 2 + scq
                    kv = h // (NQL // NKVL)
                    psp = ps.tile([128, 2 * SCW], F32, name=f"psp{sc}_{h}_{k}",
                                  tag="sc", bufs=2)
                    psps[(h, scq, k)] = psp
                    for half in range(2):
                        tcx = 2 * k + half
                        j, off, w = geom(sc, tcx)
                        qs0 = scq * SCW + off
                        diag = j >= 0
                        nc.tensor.matmul(
                            psp[:, half * SCW:half * SCW + w],
                            ksb[:, kv * S + tcx * 128:kv * S + (tcx + 1) * 128],
                            qtiles[h][:, qs0:qs0 + w],
                            start=True, stop=not diag,
                        )
                        if diag:
                            # accumulate the causal mask on PE (keeps DVE
                            # out of the attention critical path)
                            nc.tensor.matmul(
                                psp[:, half * SCW:half * SCW + 128],
                                ident[:], maskbf[:],
                                start=False, stop=True)

                def stage_b(h, scq, k):
                    # one exp ACTIVATE per pair, then DVE pre-reduction
                    sc = hf * 2 + scq
                    psp = psps.pop((h, scq, k))
                    _, _, w0 = geom(sc, 2 * k)
                    _, _, w1 = geom(sc, 2 * k + 1)
                    pt = sb.tile([128, 2 * SCW], BF16, name=f"pt{sc}_{h}_{k}",
                                 tag="pt", bufs=2)
                    if MERGE_EXP and w0 == SCW:
                        # contiguous 2D span over both halves (only when the
                        # first half is fully written - no junk-gap reads)
                        nc.scalar.activation(
                            pt[:, 0:SCW + w1], psp[:, 0:SCW + w1],
                            mybir.ActivationFunctionType.Exp, scale=SCALE)
                    else:
                        nc.scalar.activation(
                            pt[:, 0:w0], psp[:, 0:w0],
                            mybir.ActivationFunctionType.Exp, scale=SCALE)
                        nc.scalar.activation(
                            pt[:, SCW:SCW + w1], psp[:, SCW:SCW + w1],
                            mybir.ActivationFunctionType.Exp, scale=SCALE)
                    pts[(h, scq, k)] = pt
                    if PREREDUCE and w0 == SCW and w1 == SCW:
                        tmp = sb.tile([128, SCW], BF16, name=f"tm{sc}_{h}_{k}",
                                      tag="tmp", bufs=2)
                        nc.vector.tensor_tensor(
                            out=tmp[:], in0=pt[:, 0:SCW], in1=pt[:, SCW:2 * SCW],
                            op=mybir.AluOpType.add)
                        tmps[(h, scq, k)] = tmp

                def stage_c(h, scq, k):
                    # PV matmuls for both tiles of the pair
                    sc = hf * 2 + scq
                    ntc = 4 * sc + 4
                    kv = h // (NQL // NKVL)
                    if k == 0:
                        # po and pr alternate through the same 2-buf tag
                        pos[(h, scq)] = ps.tile([128, SCW], F32,
                                                name=f"po{sc}_{h}", tag="o",
                                                bufs=2)
                        prs[(h, scq)] = ps.tile([128, SCW], F32,
                                                name=f"pr{sc}_{h}", tag="o",
                                                bufs=2)
                    po = pos[(h, scq)]
                    pt = pts[(h, scq, k)]
                    for half in range(2):
                        tcx = 2 * k + half
                        j, off, w = geom(sc, tcx)
                        slot = tcx * NKVL + kv
                        nc.tensor.matmul(
                            po[:, off:], vsb[:, slot * HD:(slot + 1) * HD],
                            pt[:, half * SCW:half * SCW + w],
                            start=(tcx == 0), stop=(tcx == ntc - 1))

                def stage_d(h, scq, k):
                    # rowsum matmul(s) for the pair; normalize at group end
                    sc = hf * 2 + scq
                    npr = 2 * sc + 2
                    pr = prs[(h, scq)]
                    pt = pts.pop((h, scq, k))
                    tmp = tmps.pop((h, scq, k), None)
                    if tmp is not None:
                        nc.tensor.matmul(
                            pr[:], ones[:], tmp[:],
                            start=(k == 0), stop=False)
                    else:
                        for half in range(2):
                            tcx = 2 * k + half
                            j, off, w = geom(sc, tcx)
                            nc.tensor.matmul(
                                pr[:, off:], ones[:], pt[:, half * SCW:half * SCW + w],
                                start=(k == 0 and half == 0),
                                stop=(k == npr - 1 and half == 1))
                    if k == npr - 1:
                        po = pos.pop((h, scq))
                        prs.pop((h, scq))
                        rec = sb.tile([128, SCW], F32, name=f"rec{sc}_{h}",
                                      tag="rec", bufs=1)
                        nc.vector.reciprocal_approx_fast(out=rec[:], in_=pr[:])
                        if DEBUG_DUMP and h == 0:
                            pcp = sb.tile([128, SCW], F32, name=f"pcp{sc}",
                                          tag="dbgp", bufs=1)
                            nc.vector.tensor_copy(out=pcp[:], in_=po[:])
                            nc.sync.dma_start(
                                out=dbg["dbg_po"][:, sc * SCW:(sc + 1) * SCW],
                                in_=pcp[:])
                            nc.sync.dma_start(
                                out=dbg["dbg_rec"][:, sc * SCW:(sc + 1) * SCW],
                                in_=rec[:])
                        nc.vector.tensor_tensor(
                            out=attnT[h][:, sc * SCW:(sc + 1) * SCW],
                            in0=po[:], in1=rec[:],
                            op=mybir.AluOpType.mult)

                for i in range(n + 2):
                    if i < n:
                        stage_a(*pairs[i])
                    if 0 <= i - 1 < n:
                        stage_b(*pairs[i - 1])
                    if filler is not None and i >= 2:
                        fill_acc[0] += fill_rate
                        while fill_acc[0] >= 1.0:
                            if next(filler, None) is None:
                                filler = None
                                break
                            fill_acc[0] -= 1.0
                    if 0 <= i - 2 < n:
                        stage_c(*pairs[i - 2])
                        stage_d(*pairs[i - 2])

            # ================= main schedule =================
            wo0 = None
            for hf in range(2):
                qtiles = [None] * NQL
                if hf == 0:
                    # grouped pass over the 4 K/V heads x both s-chunks
                    grp = list(range(NQL, NM))
                    wsl4 = {m: wsls[(0, m)] for m in grp}
                    accums = []
                    for wi in range(2):
                        accums.append(ps.tile([128, SCW], F32, name=f"gp{wi}",
                                              tag="proj", bufs=2))
                    for wi in range(2):
                        gsc = ps.tile([128, 2 * SCW], F32, name=f"gsc{wi}",
                                      tag="sc", bufs=2)
                        accums.append(gsc[:, 0:SCW])
                        accums.append(gsc[:, SCW:2 * SCW])
                    for wi in range(2):
                        accums.append(ps.tile([128, SCW], F32, name=f"go{wi}",
                                              tag="o", bufs=2))
                    pp8 = {}
                    for gi, (m, scq) in enumerate(
                            (m, s) for m in grp for s in range(2)):
                        pp8[(m, scq)] = accums[gi]
                    for kc in range(NKC):
                        for m in grp:
                            for scq in range(2):
                                nc.tensor.matmul(
                                    pp8[(m, scq)],
                                    wsl4[m][:, kc * 128:(kc + 1) * 128],
                                    xts[(0, kc)][:, scq * SCW:(scq + 1) * SCW],
                                    start=(kc == 0), stop=(kc == NKC - 1),
                                )
                    # K ropes first, then free ALL V accumulators via ACT
                    # copies before any pv transpose reuses their sc slots
                    for m in (NQL, NQL + 1):
                        for scq in range(2):
                            rope_or_v(0, m, scq, pp8[(m, scq)], qtiles)
                    gvts = {}
                    for m in (NQL + NKVL, NQL + NKVL + 1):
                        for scq in range(2):
                            kv = m - NQL - NKVL
                            vts = sb.tile([128, SCW], BF16,
                                          name=f"gv{kv}_{scq}", tag="vts",
                                          bufs=4)
                            nc.scalar.copy(out=vts[:], in_=pp8[(m, scq)])
                            gvts[(kv, scq)] = vts
                    for (kv, scq), vts in gvts.items():
                        transpose_v(0, kv, scq, vts)
                    morder = list(range(NQL))
                else:
                    # K heads already projected via the hf0-attention
                    # filler; V heads then q heads remain
                    morder = [NQL + NKVL, NQL + NKVL + 1] + list(range(NQL))
                for m in morder:
                    wsl = load_wsl(hf, m)
                    if m < NQL:
                        qt = sb.tile([128, HW], BF16, name=f"q{hf}_{m}",
                                     tag=f"q{m}", bufs=1)
                        qtiles[m] = qt
                    for scq in range(2):
                        qsl = slice(scq * SCW, (scq + 1) * SCW)
                        pp = ps.tile([128, SCW], F32, name=f"pp{hf}_{m}_{scq}",
                                     tag="proj", bufs=2)
                        for kc in range(NKC):
                            nc.tensor.matmul(
                                pp[:], wsl[:, kc * 128:(kc + 1) * 128],
                                xts[(hf, kc)][:, qsl],
                                start=(kc == 0), stop=(kc == NKC - 1),
                            )
                        rope_or_v(hf, m, scq, pp, qtiles)

                if hf == 0:
                    # hf1 K-head weights first so the attention filler's
                    # first matmuls aren't stuck behind the x DMA queue
                    load_wsl(1, NQL)
                    load_wsl(1, NQL + 1)
                    load_xt(1)
                    if DEBUG_DUMP:
                        nc.sync.dma_start(out=dbg["dbg_q0"][:, :], in_=qtiles[0][:])

                    def kv1_gen():
                        # hf1 K-head projections, injected into hf0 attn
                        for m in (NQL, NQL + 1):
                            wsl1 = wsls[(1, m)]
                            for scq in range(2):
                                pp1 = ps.tile([128, SCW], F32,
                                              name=f"kp{m}_{scq}",
                                              tag="proj", bufs=2)
                                for kc in range(NKC):
                                    nc.tensor.matmul(
                                        pp1[:],
                                        wsl1[:, kc * 128:(kc + 1) * 128],
                                        xts[(1, kc)][:, scq * SCW:(scq + 1) * SCW],
                                        start=(kc == 0), stop=(kc == NKC - 1),
                                    )
                                    yield
                                rope_or_v(1, m, scq, pp1, qtiles)

                    kv1 = kv1_gen()
                    attention_half(0, qtiles, kv1, 2.0)
                    for _ in kv1:
                        pass
                else:
                    wo0 = wo_gen(0, 0, S // 256, last_pass=False)
                    attention_half(1, qtiles, wo0, 1.5)

            # drain remaining wo work: rest of first half, then second half
            wo_drain[0] = True
            for _ in wo0:
                pass
            for _ in wo_gen(1, S // 256, S // 128, last_pass=True):
                pass
            if DEBUG_DUMP:
                nc.sync.dma_start(out=dbg["dbg_ksb"][:, :], in_=ksb[:])
                nc.sync.dma_start(out=dbg["dbg_vsb"][:, :], in_=vsb[:])
                nc.sync.dma_start(out=dbg["dbg_at0"][:, :], in_=attnT[0][:])
    nc.finalize()
    return nc


_NC_CACHE = None


def _get_graph():
    global _NC_CACHE
    if _NC_CACHE is None:
        _NC_CACHE = _build()
    return _NC_CACHE


_PERM = np.concatenate([np.arange(0, HD, 2), np.arange(1, HD, 2)])


def _tile_w(w):
    """[D, M*HD] -> [128, m-major kc-major 128cols] contiguous tiling (bf16)."""
    d, mc = w.shape
    nm = mc // HD
    t = w.reshape(NKC, 128, nm, HD).transpose(1, 2, 0, 3)
    return np.ascontiguousarray(t.reshape(128, nm * NKC * HD).astype(NPBF))


def _tile_wo(w):
    """[NQL*HD, D] -> [128, dc-major kc-major 512cols] (bf16)."""
    t = w.reshape(NQL, 128, D // SCW, SCW).transpose(1, 2, 0, 3)
    return np.ascontiguousarray(
        t.reshape(128, (D // SCW) * NQL * SCW).astype(NPBF))


def _shard_inputs(x, freqs_cos, freqs_sin, wq, wk, wv, wo):
    """Build the 8 per-core input maps (pure numpy prep, nothing on-device)."""
    x = np.asarray(x, dtype=np.float32)
    wq = np.asarray(wq, dtype=np.float32)
    wk = np.asarray(wk, dtype=np.float32)
    wv = np.asarray(wv, dtype=np.float32)
    wo = np.asarray(wo, dtype=np.float32)
    cos = np.asarray(freqs_cos, dtype=np.float32)
    sin = np.asarray(freqs_sin, dtype=np.float32)

    # RoPE tables in on-chip [128, S] layout
    cos2 = np.empty((128, S), np.float32)
    sin2n = np.empty((128, S), np.float32)
    cos2[0:64] = cos.T
    cos2[64:128] = cos.T
    sin2n[0:64] = -sin.T
    sin2n[64:128] = sin.T
    cos2 = cos2.astype(NPBF)
    sin2n = sin2n.astype(NPBF)

    wq4 = wq.reshape(D, NH, HD)
    wk4 = wk.reshape(D, NKV, HD)
    wv4 = wv.reshape(D, NKV, HD)
    wo4 = wo.reshape(NH, HD, D)

    # x transposed + bf16 per batch
    xts = [np.ascontiguousarray(x[b].T.astype(NPBF)) for b in range(B)]

    in_maps = []
    for c in range(NCORES):
        b, g = divmod(c, TPG)
        qh = slice(g * NQL, (g + 1) * NQL)
        kvh = slice(g * NKVL, (g + 1) * NKVL)
        m = {
            "xt": xts[b],
            "wq": _tile_w(wq4[:, qh, :][:, :, _PERM].reshape(D, NQL * HD)),
            "wk": _tile_w(wk4[:, kvh, :][:, :, _PERM].reshape(D, NKVL * HD)),
            "wv": _tile_w(wv4[:, kvh, :].reshape(D, NKVL * HD)),
            "wo": _tile_wo(wo4[qh].reshape(NQL * HD, D)),
            "cosp": cos2,
            "sinp": sin2n,
        }
        in_maps.append(m)
    return in_maps


def kernel(x, start_pos, freqs_cos, freqs_sin, mask, wq, wk, wv, wo,
           cache_k, cache_v):
    x = np.asarray(x)
    in_maps = _shard_inputs(x, freqs_cos, freqs_sin, wq, wk, wv, wo)
    nc = _get_graph()
    res = run_bass_kernel_spmd(nc, in_maps, core_ids=list(range(NCORES)))
    out = np.zeros((B, S, D), dtype=np.float32)
    for b in range(B):
        acc = np.asarray(res.results[b * TPG]["out"]).astype(np.float32)
        for g in range(1, TPG):
            acc += np.asarray(res.results[b * TPG + g]["out"]).astype(np.float32)
        out[b] = acc
    return out


# revision 40
# speedup vs baseline: 1.1799x; 1.1799x over previous
"""Distributed Trainium2 Bass kernel for GQA attention prefill.

Problem: B=2, S=2048, D=4096, 32 q heads, 8 kv heads, head_dim=128, RoPE,
causal mask, start_pos=0.

Sharding (8 cores): DP2 over batch x TP4 over heads.  Core c = b*4 + g gets
batch b, q-heads 8g..8g+7, kv-heads 2g..2g+1, wo rows for those q-heads.
Each core computes a partial [S, D] output (bf16); the host sums the 4
partials per batch (the row-parallel wo unshard).

All layout work happens on the host: x arrives pre-transposed and pre-cast
to bf16, weights arrive bf16 pre-tiled, RoPE tables arrive precomputed in
their on-chip [128, S] layout.

On-core dataflow per half (1024 query positions):
  load xT bf16 tiles [128, 1024]; QKV projection (bf16 matmuls); RoPE on
  the projection PSUM.  Attention runs a pair-pipelined software pipeline:
  each PAIR of 128-key score tiles lands in one 2-bank PSUM pool tile, exp
  runs one ACTIVATE per pair (halving ACT instruction overhead), a DVE
  pre-reduction sums each full pair of P tiles so the softmax denominator
  needs one ones-matmul per pair instead of two (cuts PE rowsum cycles),
  and the causal mask is accumulated by an extra PE matmul (ident x mask)
  instead of a DVE add, keeping DVE off the critical path.  PE bubbles in
  the ACT/DVE-paced attention phases are filled with injected matmuls:
  the next half's K-head projections during the first half's attention,
  and the wo output projection for s<1024 during the second half's.  The
  remaining wo work runs as a tail (wo streamed once per s-half), with
  PSUM->SBUF copies on ACT and output DMAs alternating rings.
"""

import math

import numpy as np
import ml_dtypes

import concourse.bass as bass  # noqa: F401  (bass types via bacc)
import concourse.mybir as mybir
from concourse import bacc
from concourse.bass_utils import run_bass_kernel_spmd
from concourse.tile import TileContext

F32 = mybir.dt.float32
BF16 = mybir.dt.bfloat16
NPBF = ml_dtypes.bfloat16

B, S, D = 2, 2048, 4096
NH, NKV, HD = 32, 8, 128
NCORES = 8
TPG = 4                  # tensor-parallel groups
NQL = NH // TPG          # 8 local q heads
NKVL = NKV // TPG        # 2 local kv heads
SCW = 512                # s-chunk width
HW = S // 2              # half width (1024)
NKC = D // 128           # 32 contraction chunks for projections
NTC = S // 128           # 16 T-chunks (key positions)
SCALE = 1.0 / math.sqrt(HD)
NEG = -1e9
MERGE_EXP = True      # one ACTIVATE per adjacent-slot score pair
PREREDUCE = True      # DVE pair pre-reduction + single rowsum matmul
DEBUG_DUMP = False    # add debug DRAM outputs (sim bisection only)


def _build():
    nc = bacc.Bacc("TRN2", target_bir_lowering=False, debug=False,
                   num_devices=NCORES)
    # x pre-transposed + pre-cast: [D, S] bf16
    xt_d = nc.declare_dram_parameter("xt", [D, S], BF16, isOutput=False)
    # weights arrive pre-tiled bf16: [128, m-major kc-major cols]
    wq = nc.declare_dram_parameter("wq", [128, NQL * NKC * HD], BF16, isOutput=False)
    wk = nc.declare_dram_parameter("wk", [128, NKVL * NKC * HD], BF16, isOutput=False)
    wv = nc.declare_dram_parameter("wv", [128, NKVL * NKC * HD], BF16, isOutput=False)
    wo = nc.declare_dram_parameter("wo", [128, (D // SCW) * NQL * SCW], BF16, isOutput=False)
    # RoPE tables in on-chip layout [128, S]
    cosp = nc.declare_dram_parameter("cosp", [128, S], BF16, isOutput=False)
    sinp = nc.declare_dram_parameter("sinp", [128, S], BF16, isOutput=False)
    out = nc.declare_dram_parameter("out", [S, D], BF16, isOutput=True)
    if DEBUG_DUMP:
        dbg = {
            "dbg_ksb": nc.declare_dram_parameter("dbg_ksb", [128, NKVL * S], BF16, isOutput=True),
            "dbg_vsb": nc.declare_dram_parameter("dbg_vsb", [128, NTC * NKVL * HD], BF16, isOutput=True),
            "dbg_q0": nc.declare_dram_parameter("dbg_q0", [128, HW], BF16, isOutput=True),
            "dbg_at0": nc.declare_dram_parameter("dbg_at0", [128, S], BF16, isOutput=True),
            "dbg_rec": nc.declare_dram_parameter("dbg_rec", [128, 4 * SCW], F32, isOutput=True),
            "dbg_po": nc.declare_dram_parameter("dbg_po", [128, 4 * SCW], F32, isOutput=True),
        }

    NM = NQL + 2 * NKVL

    with TileContext(nc) as tc:
        with (
            tc.tile_pool(name="const", bufs=1) as const,
            tc.tile_pool(name="big", bufs=1) as big,
            tc.tile_pool(name="sb", bufs=3) as sb,
            tc.tile_pool(name="ps", bufs=1, space="PSUM") as ps,
        ):
            # ---- weight-slice loader ----
            wsls = {}

            def load_wsl(hf, m):
                if (hf, m) in wsls:
                    return wsls[(hf, m)]
                wsl = sb.tile([128, NKC * HD], BF16, name=f"w{hf}_{m}",
                              tag="wsl", bufs=4)
                if m < NQL:
                    base = wq
                    m0 = m
                elif m < NQL + NKVL:
                    base = wk
                    m0 = m - NQL
                else:
                    base = wv
                    m0 = m - NQL - NKVL
                qw = NKC * HD // 4
                for q4 in range(4):
                    c0 = m0 * NKC * HD + q4 * qw
                    # split the weight stream across two rings so the
                    # Q-head loops never catch up with a clogged DMA queue;
                    # hf0 Q-heads use sync (not scalar) so their descriptor
                    # issues don't sit ahead of the attention exps in the
                    # ACT queue
                    if q4 % 2 == 0:
                        eng = nc.gpsimd
                    elif hf == 0 and m < NQL:
                        eng = nc.sync
                    else:
                        eng = nc.scalar
                    eng.dma_start(out=wsl[:, q4 * qw:(q4 + 1) * qw],
                                  in_=base[:, c0:c0 + qw])
                wsls[(hf, m)] = wsl
                return wsl

            # K/V head weights for the grouped pass: interleave the four
            # column-quarters across heads so every head's first quarter
            # lands before any head's second quarter
            for m0_ in range(NQL, NQL + 2 * NKVL):
                wsls[(0, m0_)] = sb.tile([128, NKC * HD], BF16,
                                         name=f"w0_{m0_}", tag="wsl", bufs=4)
            qw0 = NKC * HD // 4
            for q4 in range(4):
                for m0_ in range(NQL, NQL + 2 * NKVL):
                    if m0_ < NQL + NKVL:
                        base0 = wk
                        mb0 = m0_ - NQL
                    else:
                        base0 = wv
                        mb0 = m0_ - NQL - NKVL
                    c0 = mb0 * NKC * HD + q4 * qw0
                    eng0 = nc.scalar if q4 % 2 == 1 else nc.gpsimd
                    eng0.dma_start(
                        out=wsls[(0, m0_)][:, q4 * qw0:(q4 + 1) * qw0],
                        in_=base0[:, c0:c0 + qw0])

            # ---- constants ----
            ident = const.tile([128, 128], BF16, name="ident")
            nc.gpsimd.memset(ident[:], 0.0)
            nc.gpsimd.affine_select(
                out=ident[:], in_=ident[:],
                compare_op=mybir.AluOpType.not_equal, fill=1.0,
                base=0, pattern=[[-1, 128]], channel_multiplier=1,
            )
            ones = const.tile([128, 128], BF16, name="ones")
            nc.gpsimd.memset(ones[:], 1.0)
            # causal mask as a bf16 matmul operand: applied to the score
            # PSUM via an accumulating PE matmul (keeps the mask off DVE)
            maskbf = const.tile([128, 128], BF16, name="maskbf")
            nc.gpsimd.memset(maskbf[:], 0.0)
            nc.gpsimd.affine_select(
                out=maskbf[:], in_=maskbf[:],
                compare_op=mybir.AluOpType.is_ge, fill=NEG,
                base=0, pattern=[[1, 128]], channel_multiplier=-1,
            )
            # RoPE tables ride the sync ring (empty at start) so the PE
            # warmup matmuls aren't stuck behind the weight-quarter stream
            cos2 = const.tile([128, S], BF16, name="cos2")
            sin2n = const.tile([128, S], BF16, name="sin2n")
            for hh in range(2):
                hsl = slice(hh * (S // 2), (hh + 1) * (S // 2))
                nc.sync.dma_start(out=cos2[:, hsl], in_=cosp[:, hsl])
                nc.sync.dma_start(out=sin2n[:, hsl], in_=sinp[:, hsl])

            # PE p-state warmup: dummy matmuls ramp the array to full clock
            # (touch every PSUM tag: 2 proj + 2 sc pair-tiles + 2 o)
            warm_dsts = []
            for wi in range(2):
                warm_dsts.append(ps.tile([128, SCW], F32, name=f"pwp{wi}",
                                         tag="proj", bufs=2))
            for wi in range(2):
                wsc = ps.tile([128, 2 * SCW], F32, name=f"pws{wi}",
                              tag="sc", bufs=2)
                warm_dsts.append(wsc[:, 0:SCW])
                warm_dsts.append(wsc[:, SCW:2 * SCW])
            for wi in range(2):
                warm_dsts.append(ps.tile([128, SCW], F32, name=f"pwo{wi}",
                                         tag="o", bufs=2))
            ksb = big.tile([128, NKVL * S], BF16, name="ksb")
            vsb = big.tile([128, NTC * NKVL * HD], BF16, name="vsb")
            # attention outputs for the full sequence
            attnT = [big.tile([128, S], BF16, name=f"at{h}") for h in range(NQL)]

            # warmup copies scribble into attnT (overwritten later)
            for wi, pw in enumerate(warm_dsts):
                nc.tensor.matmul(pw, cos2[:, 0:128], cos2[:, 0:SCW],
                                 start=True, stop=True)
                nc.vector.tensor_copy(
                    out=attnT[wi // 4][:, (wi % 4) * SCW:(wi % 4 + 1) * SCW],
                    in_=pw)

            # ---- xT loads: per (hf, kc) one tile [128, 1024] ----
            xts = {}

            def load_xt(hf):
                for kc in range(NKC):
                    t = sb.tile([128, HW], BF16, name=f"xt{hf}_{kc}",
                                tag="xt", bufs=32)
                    if kc % 2 == 0:
                        eng = nc.sync
                    elif hf == 0:
                        eng = nc.scalar
                    else:
                        eng = nc.gpsimd
                    eng.dma_start(
                        out=t[:],
                        in_=xt_d[kc * 128:(kc + 1) * 128, hf * HW:(hf + 1) * HW])
                    xts[(hf, kc)] = t

            load_xt(0)

            def rope_or_v(hf, m, scq, pp, qtiles):
                sc = hf * 2 + scq
                ssl = slice(sc * SCW, (sc + 1) * SCW)
                qsl = slice(scq * SCW, (scq + 1) * SCW)
                if m < NQL + NKVL:
                    if m < NQL:
                        dst = qtiles[m][:, qsl]
                    else:
                        kv = m - NQL
                        dst = ksb[:, kv * S + sc * SCW:kv * S + (sc + 1) * SCW]
                    t1 = sb.tile([128, SCW], BF16, name=f"t1_{hf}_{m}_{scq}",
                                 tag="t1", bufs=2)
                    t2 = sb.tile([128, SCW], BF16, name=f"t2_{hf}_{m}_{scq}",
                                 tag="t2", bufs=2)
                    nc.vector.tensor_tensor(
                        out=t1[0:64, :], in0=pp[64:128, :],
                        in1=sin2n[0:64, ssl], op=mybir.AluOpType.mult)
                    nc.vector.tensor_tensor(
                        out=t1[64:128, :], in0=pp[0:64, :],
                        in1=sin2n[64:128, ssl], op=mybir.AluOpType.mult)
                    nc.vector.tensor_tensor(
                        out=t2[:], in0=pp[:], in1=cos2[:, ssl],
                        op=mybir.AluOpType.mult)
                    nc.vector.tensor_tensor(
                        out=dst, in0=t1[:], in1=t2[:],
                        op=mybir.AluOpType.add)
                else:
                    kv = m - NQL - NKVL
                    vts = sb.tile([128, SCW], BF16, name=f"vts{hf}_{kv}_{scq}",
                                  tag="vts", bufs=4)
                    nc.scalar.copy(out=vts[:], in_=pp)
                    transpose_v(hf, kv, scq, vts)

            def transpose_v(hf, kv, scq, vts):
                sc = hf * 2 + scq
                for j in range(SCW // 128):
                    pv = ps.tile([128, 128], BF16,
                                 name=f"pv{hf}_{kv}_{scq}_{j}",
                                 tag="sc", bufs=2)
                    nc.tensor.transpose(
                        pv[:], vts[:, j * 128:(j + 1) * 128], ident[:])
                    slot = (sc * 4 + j) * NKVL + kv
                    nc.scalar.copy(
                        out=vsb[:, slot * HD:(slot + 1) * HD], in_=pv[:])

            # ---- wo output projection as a resumable generator ----------
            oblk = NQL * SCW
            wots = {}

            def load_wot(li, dc):
                wot = sb.tile([128, NQL * SCW], BF16, name=f"wot{li}_{dc}",
                              tag="wot", bufs=2)
                qw = oblk // 4
                for q4 in range(4):
                    nc.gpsimd.dma_start(
                        out=wot[:, q4 * qw:(q4 + 1) * qw],
                        in_=wo[:, dc * oblk + q4 * qw:dc * oblk + (q4 + 1) * qw])
                wots[(li, dc)] = wot

            # after the attention phases complete, the sc/o PSUM banks are
            # idle: drain-mode wo accumulators cycle through all three tags
            # (6-deep pipeline) so dc-boundary os-copy chains never stall PE
            wo_drain = [False]

            def wo_gen(li, ssub_lo, ssub_hi, last_pass):
                """Yields once per wo matmul; boundary ops emitted inline."""
                load_wot(li, 0)
                load_wot(li, 1)
                cnt = 0
                for dc in range(D // SCW):
                    wot = wots[(li, dc)]
                    if dc + 2 < D // SCW:
                        load_wot(li, dc + 2)
                    for ssub in range(ssub_lo, ssub_hi):
                        s0 = ssub * 128
                        tg = ("proj", "sc", "o")[cnt % 3] if wo_drain[0] else "proj"
                        pd = ps.tile([128, SCW], F32, name=f"pd{li}_{dc}_{ssub}",
                                     tag=tg, bufs=2)
                        for kc8 in range(NQL):
                            nc.tensor.matmul(
                                pd[:],
                                attnT[kc8][:, s0:s0 + 128],
                                wot[:, kc8 * SCW:(kc8 + 1) * SCW],
                                start=(kc8 == 0), stop=(kc8 == NQL - 1))
                            yield
                        os_ = sb.tile([128, SCW], BF16, name=f"os{li}_{dc}_{ssub}",
                                      tag="os", bufs=4)
                        if wo_drain[0] and cnt % 2 == 0:
                            nc.scalar.copy(out=os_[:], in_=pd[:])
                        else:
                            nc.vector.tensor_copy(out=os_[:], in_=pd[:])
                        if last_pass and dc == D // SCW - 1:
                            oeng = (nc.sync, nc.gpsimd, nc.scalar)[ssub % 3]
                        else:
                            oeng = nc.sync if ssub % 2 == 0 else nc.gpsimd
                        oeng.dma_start(
                            out=out[s0:s0 + 128, dc * SCW:(dc + 1) * SCW],
                            in_=os_[:])
                        cnt += 1

            # ---- attention for one half: pair-pipelined ------------------
            def attention_half(hf, qtiles, filler, fill_rate):
                pairs = []
                for h in range(NQL):
                    for scq in range(2):
                        sc = hf * 2 + scq
                        for k in range(2 * sc + 2):
                            pairs.append((h, scq, k))
                n = len(pairs)
                psps = {}    # (h, scq, k) -> score pair tile [128, 1024] PSUM
                pts = {}     # (h, scq, k) -> pt pair tile [128, 1024]
                tmps = {}    # (h, scq, k) -> prereduced tile (full pairs)
                pos = {}
                prs = {}
                fill_acc = [0.0]

                def geom(sc, tcx):
                    j = tcx - 4 * sc
                    off = j * 128 if j > 0 else 0
                    return j, off, SCW - off

                def stage_a(h, scq, k):
                    # scores for both tiles of the pair + diagonal masks
                    sc = hf * 2 + scq
                    kv = h // (NQL // NKVL)
                    psp = ps.tile([128, 2 * SCW], F32, name=f"psp{sc}_{h}_{k}",
                                  tag="sc", bufs=2)
                    psps[(h, scq, k)] = psp
                    for half in range(2):
                        tcx = 2 * k + half
                        j, off, w = geom(sc, tcx)
                        qs0 = scq * SCW + off
                        diag = j >= 0
                        nc.tensor.matmul(
                            psp[:, half * SCW:half * SCW + w],
                            ksb[:, kv * S + tcx * 128:kv * S + (tcx + 1) * 128],
                            qtiles[h][:, qs0:qs0 + w],
                            start=True, stop=not diag,
                        )
                        if diag:
                            # accumulate the causal mask on PE (keeps DVE
                            # out of the attention critical path)
                            nc.tensor.matmul(
                                psp[:, half * SCW:half * SCW + 128],
                                ident[:], maskbf[:],
                                start=False, stop=True)

                def stage_b(h, scq, k):
                    # one exp ACTIVATE per pair, then DVE pre-reduction
                    sc = hf * 2 + scq
                    psp = psps.pop((h, scq, k))
                    _, _, w0 = geom(sc, 2 * k)
                    _, _, w1 = geom(sc, 2 * k + 1)
                    pt = sb.tile([128, 2 * SCW], BF16, name=f"pt{sc}_{h}_{k}",
                                 tag="pt", bufs=2)
                    if MERGE_EXP and w0 == SCW:
                        # contiguous 2D span over both halves (only when the
                        # first half is fully written - no junk-gap reads)
                        nc.scalar.activation(
                            pt[:, 0:SCW + w1], psp[:, 0:SCW + w1],
                            mybir.ActivationFunctionType.Exp, scale=SCALE)
                    else:
                        nc.scalar.activation(
                            pt[:, 0:w0], psp[:, 0:w0],
                            mybir.ActivationFunctionType.Exp, scale=SCALE)
                        nc.scalar.activation(
                            pt[:, SCW:SCW + w1], psp[:, SCW:SCW + w1],
                            mybir.ActivationFunctionType.Exp, scale=SCALE)
                    pts[(h, scq, k)] = pt
                    if PREREDUCE and w0 == SCW and w1 == SCW:
                        tmp = sb.tile([128, SCW], BF16, name=f"tm{sc}_{h}_{k}",
                                      tag="tmp", bufs=2)
                        nc.vector.tensor_tensor(
                            out=tmp[:], in0=pt[:, 0:SCW], in1=pt[:, SCW:2 * SCW],
                            op=mybir.AluOpType.add)
                        tmps[(h, scq, k)] = tmp

                def stage_c(h, scq, k):
                    # PV matmuls for both tiles of the pair
                    sc = hf * 2 + scq
                    ntc = 4 * sc + 4
                    kv = h // (NQL // NKVL)
                    if k == 0:
                        # po and pr alternate through the same 2-buf tag
                        pos[(h, scq)] = ps.tile([128, SCW], F32,
                                                name=f"po{sc}_{h}", tag="o",
                                                bufs=2)
                        prs[(h, scq)] = ps.tile([128, SCW], F32,
                                                name=f"pr{sc}_{h}", tag="o",
                                                bufs=2)
                    po = pos[(h, scq)]
                    pt = pts[(h, scq, k)]
                    for half in range(2):
                        tcx = 2 * k + half
                        j, off, w = geom(sc, tcx)
                        slot = tcx * NKVL + kv
                        nc.tensor.matmul(
                            po[:, off:], vsb[:, slot * HD:(slot + 1) * HD],
                            pt[:, half * SCW:half * SCW + w],
                            start=(tcx == 0), stop=(tcx == ntc - 1))

                def stage_d(h, scq, k):
                    # rowsum matmul(s) for the pair; normalize at group end
                    sc = hf * 2 + scq
                    npr = 2 * sc + 2
                    pr = prs[(h, scq)]
                    pt = pts.pop((h, scq, k))
                    tmp = tmps.pop((h, scq, k), None)
                    if tmp is not None:
                        nc.tensor.matmul(
                            pr[:], ones[:], tmp[:],
                            start=(k == 0), stop=False)
                    else:
                        for half in range(2):
                            tcx = 2 * k + half
                            j, off, w = geom(sc, tcx)
                            nc.tensor.matmul(
                                pr[:, off:], ones[:], pt[:, half * SCW:half * SCW + w],
                                start=(k == 0 and half == 0),
                                stop=(k == npr - 1 and half == 1))
                    if k == npr - 1:
                        po = pos.pop((h, scq))
                        prs.pop((h, scq))
                        rec = sb.tile([128, SCW], F32, name=f"rec{sc}_{h}",
                                      tag="rec", bufs=1)
                        nc.vector.reciprocal_approx_fast(out=rec[:], in_=pr[:])
                        if DEBUG_DUMP and h == 0:
                            pcp = sb.tile([128, SCW], F32, name=f"pcp{sc}",
                                          tag="dbgp", bufs=1)
                            nc.vector.tensor_copy(out=pcp[:], in_=po[:])
                            nc.sync.dma_start(
                                out=dbg["dbg_po"][:, sc * SCW:(sc + 1) * SCW],
                                in_=pcp[:])
                            nc.sync.dma_start(
                                out=dbg["dbg_rec"][:, sc * SCW:(sc + 1) * SCW],
                                in_=rec[:])
                        nc.vector.tensor_tensor(
                            out=attnT[h][:, sc * SCW:(sc + 1) * SCW],
                            in0=po[:], in1=rec[:],
                            op=mybir.AluOpType.mult)

                for i in range(n + 2):
                    if i < n:
                        stage_a(*pairs[i])
                    if 0 <= i - 1 < n:
                        stage_b(*pairs[i - 1])
                    if filler is not None and i >= 2:
                        fill_acc[0] += fill_rate
                        while fill_acc[0] >= 1.0:
                            if next(filler, None) is None:
                                filler = None
                                break
                            fill_acc[0] -= 1.0
                    if 0 <= i - 2 < n:
                        stage_c(*pairs[i - 2])
                        stage_d(*pairs[i - 2])

            # ================= main schedule =================
            wo0 = None
            for hf in range(2):
                qtiles = [None] * NQL
                if hf == 0:
                    # grouped pass over the 4 K/V heads x both s-chunks
                    grp = list(range(NQL, NM))
                    wsl4 = {m: wsls[(0, m)] for m in grp}
                    accums = []
                    for wi in range(2):
                        accums.append(ps.tile([128, SCW], F32, name=f"gp{wi}",
                                              tag="proj", bufs=2))
                    for wi in range(2):
                        gsc = ps.tile([128, 2 * SCW], F32, name=f"gsc{wi}",
                                      tag="sc", bufs=2)
                        accums.append(gsc[:, 0:SCW])
                        accums.append(gsc[:, SCW:2 * SCW])
                    for wi in range(2):
                        accums.append(ps.tile([128, SCW], F32, name=f"go{wi}",
                                              tag="o", bufs=2))
                    pp8 = {}
                    for gi, (m, scq) in enumerate(
                            (m, s) for m in grp for s in range(2)):
                        pp8[(m, scq)] = accums[gi]
                    for kc in range(NKC):
                        for m in grp:
                            for scq in range(2):
                                nc.tensor.matmul(
                                    pp8[(m, scq)],
                                    wsl4[m][:, kc * 128:(kc + 1) * 128],
                                    xts[(0, kc)][:, scq * SCW:(scq + 1) * SCW],
                                    start=(kc == 0), stop=(kc == NKC - 1),
                                )
                    # K ropes first, then free ALL V accumulators via ACT
                    # copies before any pv transpose reuses their sc slots
                    for m in (NQL, NQL + 1):
                        for scq in range(2):
                            rope_or_v(0, m, scq, pp8[(m, scq)], qtiles)
                    gvts = {}
                    for m in (NQL + NKVL, NQL + NKVL + 1):
                        for scq in range(2):
                            kv = m - NQL - NKVL
                            vts = sb.tile([128, SCW], BF16,
                                          name=f"gv{kv}_{scq}", tag="vts",
                                          bufs=4)
                            nc.scalar.copy(out=vts[:], in_=pp8[(m, scq)])
                            gvts[(kv, scq)] = vts
                    for (kv, scq), vts in gvts.items():
                        transpose_v(0, kv, scq, vts)
                    morder = list(range(NQL))
                else:
                    # K heads already projected via the hf0-attention
                    # filler; V heads then q heads remain
                    morder = [NQL + NKVL, NQL + NKVL + 1] + list(range(NQL))
                for m in morder:
                    wsl = load_wsl(hf, m)
                    if m < NQL:
                        qt = sb.tile([128, HW], BF16, name=f"q{hf}_{m}",
                                     tag=f"q{m}", bufs=1)
                        qtiles[m] = qt
                    for scq in range(2):
                        qsl = slice(scq * SCW, (scq + 1) * SCW)
                        pp = ps.tile([128, SCW], F32, name=f"pp{hf}_{m}_{scq}",
                                     tag="proj", bufs=2)
                        for kc in range(NKC):
                            nc.tensor.matmul(
                                pp[:], wsl[:, kc * 128:(kc + 1) * 128],
                                xts[(hf, kc)][:, qsl],
                                start=(kc == 0), stop=(kc == NKC - 1),
                            )
                        rope_or_v(hf, m, scq, pp, qtiles)

                if hf == 0:
                    # hf1 K-head weights first so the attention filler's
                    # first matmuls aren't stuck behind the x DMA queue
                    load_wsl(1, NQL)
                    load_wsl(1, NQL + 1)
                    load_xt(1)
                    if DEBUG_DUMP:
                        nc.sync.dma_start(out=dbg["dbg_q0"][:, :], in_=qtiles[0][:])

                    def kv1_gen():
                        # hf1 K-head projections, injected into hf0 attn
                        for m in (NQL, NQL + 1):
                            wsl1 = wsls[(1, m)]
                            for scq in range(2):
                                pp1 = ps.tile([128, SCW], F32,
                                              name=f"kp{m}_{scq}",
                                              tag="proj", bufs=2)
                                for kc in range(NKC):
                                    nc.tensor.matmul(
                                        pp1[:],
                                        wsl1[:, kc * 128:(kc + 1) * 128],
                                        xts[(1, kc)][:, scq * SCW:(scq + 1) * SCW],
                                        start=(kc == 0), stop=(kc == NKC - 1),
                                    )
                                    yield
                                rope_or_v(1, m, scq, pp1, qtiles)

                    kv1 = kv1_gen()
                    attention_half(0, qtiles, kv1, 2.0)
                    for _ in kv1:
                        pass
                else:
                    wo0 = wo_gen(0, 0, S // 256, last_pass=False)
                    attention_half(1, qtiles, wo0, 1.5)

            # drain remaining wo work: rest of first half, then second half
            wo_drain[0] = True
            for _ in wo0:
                pass
            for _ in wo_gen(1, S // 256, S // 128, last_pass=True):
                pass
            if DEBUG_DUMP:
                nc.sync.dma_start(out=dbg["dbg_ksb"][:, :], in_=ksb[:])
                nc.sync.dma_start(out=dbg["dbg_vsb"][:, :], in_=vsb[:])
                nc.sync.dma_start(out=dbg["dbg_at0"][:, :], in_=attnT[0][:])
    nc.finalize()
    return nc


_NC_CACHE = None


def _get_graph():
    global _NC_CACHE
    if _NC_CACHE is None:
        _NC_CACHE = _build()
    return _NC_CACHE


_PERM = np.concatenate([np.arange(0, HD, 2), np.arange(1, HD, 2)])


def _tile_w(w):
    """[D, M*HD] -> [128, m-major kc-major 128cols] contiguous tiling (bf16)."""
    d, mc = w.shape
    nm = mc // HD
    t = w.reshape(NKC, 128, nm, HD).transpose(1, 2, 0, 3)
    return np.ascontiguousarray(t.reshape(128, nm * NKC * HD).astype(NPBF))


def _tile_wo(w):
    """[NQL*HD, D] -> [128, dc-major kc-major 512cols] (bf16)."""
    t = w.reshape(NQL, 128, D // SCW, SCW).transpose(1, 2, 0, 3)
    return np.ascontiguousarray(
        t.reshape(128, (D // SCW) * NQL * SCW).astype(NPBF))


def _shard_inputs(x, freqs_cos, freqs_sin, wq, wk, wv, wo):
    """Build the 8 per-core input maps (pure numpy prep, nothing on-device)."""
    x = np.asarray(x, dtype=np.float32)
    wq = np.asarray(wq, dtype=np.float32)
    wk = np.asarray(wk, dtype=np.float32)
    wv = np.asarray(wv, dtype=np.float32)
    wo = np.asarray(wo, dtype=np.float32)
    cos = np.asarray(freqs_cos, dtype=np.float32)
    sin = np.asarray(freqs_sin, dtype=np.float32)

    # RoPE tables in on-chip [128, S] layout
    cos2 = np.empty((128, S), np.float32)
    sin2n = np.empty((128, S), np.float32)
    cos2[0:64] = cos.T
    cos2[64:128] = cos.T
    sin2n[0:64] = -sin.T
    sin2n[64:128] = sin.T
    cos2 = cos2.astype(NPBF)
    sin2n = sin2n.astype(NPBF)

    wq4 = wq.reshape(D, NH, HD)
    wk4 = wk.reshape(D, NKV, HD)
    wv4 = wv.reshape(D, NKV, HD)
    wo4 = wo.reshape(NH, HD, D)

    # x transposed + bf16 per batch
    xts = [np.ascontiguousarray(x[b].T.astype(NPBF)) for b in range(B)]

    in_maps = []
    for c in range(NCORES):
        b, g = divmod(c, TPG)
        qh = slice(g * NQL, (g + 1) * NQL)
        kvh = slice(g * NKVL, (g + 1) * NKVL)
        m = {
            "xt": xts[b],
            "wq": _tile_w(wq4[:, qh, :][:, :, _PERM].reshape(D, NQL * HD)),
            "wk": _tile_w(wk4[:, kvh, :][:, :, _PERM].reshape(D, NKVL * HD)),
            "wv": _tile_w(wv4[:, kvh, :].reshape(D, NKVL * HD)),
            "wo": _tile_wo(wo4[qh].reshape(NQL * HD, D)),
            "cosp": cos2,
            "sinp": sin2n,
        }
        in_maps.append(m)
    return in_maps


def kernel(x, start_pos, freqs_cos, freqs_sin, mask, wq, wk, wv, wo,
           cache_k, cache_v):
    x = np.asarray(x)
    in_maps = _shard_inputs(x, freqs_cos, freqs_sin, wq, wk, wv, wo)
    nc = _get_graph()
    res = run_bass_kernel_spmd(nc, in_maps, core_ids=list(range(NCORES)))
    out = np.zeros((B, S, D), dtype=np.float32)
    for b in range(B):
        acc = np.asarray(res.results[b * TPG]["out"]).astype(np.float32)
        for g in range(1, TPG):
            acc += np.asarray(res.results[b * TPG + g]["out"]).astype(np.float32)
        out[b] = acc
    return out


# revision 42
# speedup vs baseline: 1.1891x; 1.0078x over previous
"""Distributed Trainium2 Bass kernel for GQA attention prefill.

Problem: B=2, S=2048, D=4096, 32 q heads, 8 kv heads, head_dim=128, RoPE,
causal mask, start_pos=0.

Sharding (8 cores): DP2 over batch x TP4 over heads.  Core c = b*4 + g gets
batch b, q-heads 8g..8g+7, kv-heads 2g..2g+1, wo rows for those q-heads.
Each core computes a partial [S, D] output (bf16); the host sums the 4
partials per batch (the row-parallel wo unshard).

All layout work happens on the host: x arrives pre-transposed and pre-cast
to bf16, weights arrive bf16 pre-tiled, RoPE tables arrive precomputed in
their on-chip [128, S] layout.

On-core dataflow per half (1024 query positions):
  load xT bf16 tiles [128, 1024]; QKV projection (bf16 matmuls); RoPE on
  the projection PSUM.  Attention runs a pair-pipelined software pipeline:
  each PAIR of 128-key score tiles lands in one 2-bank PSUM pool tile, exp
  runs one ACTIVATE per pair (halving ACT instruction overhead), a DVE
  pre-reduction sums each full pair of P tiles so the softmax denominator
  needs one ones-matmul per pair instead of two (cuts PE rowsum cycles),
  and the causal mask is accumulated by an extra PE matmul (ident x mask)
  instead of a DVE add, keeping DVE off the critical path.  PE bubbles in
  the ACT/DVE-paced attention phases are filled with injected matmuls:
  the next half's K-head projections during the first half's attention,
  and the wo output projection for s<1024 during the second half's.  The
  remaining wo work runs as a tail (wo streamed once per s-half), with
  PSUM->SBUF copies on ACT and output DMAs alternating rings.
"""

import math

import numpy as np
import ml_dtypes

import concourse.bass as bass  # noqa: F401  (bass types via bacc)
import concourse.mybir as mybir
from concourse import bacc
from concourse.bass_utils import run_bass_kernel_spmd
from concourse.tile import TileContext

F32 = mybir.dt.float32
BF16 = mybir.dt.bfloat16
NPBF = ml_dtypes.bfloat16

B, S, D = 2, 2048, 4096
NH, NKV, HD = 32, 8, 128
NCORES = 8
TPG = 4                  # tensor-parallel groups
NQL = NH // TPG          # 8 local q heads
NKVL = NKV // TPG        # 2 local kv heads
SCW = 512                # s-chunk width
HW = S // 2              # half width (1024)
NKC = D // 128           # 32 contraction chunks for projections
NTC = S // 128           # 16 T-chunks (key positions)
SCALE = 1.0 / math.sqrt(HD)
NEG = -1e9
MERGE_EXP = True      # one ACTIVATE per adjacent-slot score pair
PREREDUCE = True      # DVE pair pre-reduction + single rowsum matmul
DEBUG_DUMP = False    # add debug DRAM outputs (sim bisection only)


def _build():
    nc = bacc.Bacc("TRN2", target_bir_lowering=False, debug=False,
                   num_devices=NCORES)
    # x pre-transposed + pre-cast: [D, S] bf16
    xt_d = nc.declare_dram_parameter("xt", [D, S], BF16, isOutput=False)
    # weights arrive pre-tiled bf16: [128, m-major kc-major cols]
    wq = nc.declare_dram_parameter("wq", [128, NQL * NKC * HD], BF16, isOutput=False)
    wk = nc.declare_dram_parameter("wk", [128, NKVL * NKC * HD], BF16, isOutput=False)
    wv = nc.declare_dram_parameter("wv", [128, NKVL * NKC * HD], BF16, isOutput=False)
    wo = nc.declare_dram_parameter("wo", [128, (D // SCW) * NQL * SCW], BF16, isOutput=False)
    # RoPE tables in on-chip layout [128, S]
    cosp = nc.declare_dram_parameter("cosp", [128, S], BF16, isOutput=False)
    sinp = nc.declare_dram_parameter("sinp", [128, S], BF16, isOutput=False)
    out = nc.declare_dram_parameter("out", [S, D], BF16, isOutput=True)
    if DEBUG_DUMP:
        dbg = {
            "dbg_ksb": nc.declare_dram_parameter("dbg_ksb", [128, NKVL * S], BF16, isOutput=True),
            "dbg_vsb": nc.declare_dram_parameter("dbg_vsb", [128, NTC * NKVL * HD], BF16, isOutput=True),
            "dbg_q0": nc.declare_dram_parameter("dbg_q0", [128, HW], BF16, isOutput=True),
            "dbg_at0": nc.declare_dram_parameter("dbg_at0", [128, S], BF16, isOutput=True),
            "dbg_rec": nc.declare_dram_parameter("dbg_rec", [128, 4 * SCW], F32, isOutput=True),
            "dbg_po": nc.declare_dram_parameter("dbg_po", [128, 4 * SCW], F32, isOutput=True),
        }

    NM = NQL + 2 * NKVL

    with TileContext(nc) as tc:
        with (
            tc.tile_pool(name="const", bufs=1) as const,
            tc.tile_pool(name="big", bufs=1) as big,
            tc.tile_pool(name="sb", bufs=3) as sb,
            tc.tile_pool(name="ps", bufs=1, space="PSUM") as ps,
        ):
            # ---- weight-slice loader ----
            wsls = {}

            def load_wsl(hf, m):
                if (hf, m) in wsls:
                    return wsls[(hf, m)]
                wsl = sb.tile([128, NKC * HD], BF16, name=f"w{hf}_{m}",
                              tag="wsl", bufs=4)
                if m < NQL:
                    base = wq
                    m0 = m
                elif m < NQL + NKVL:
                    base = wk
                    m0 = m - NQL
                else:
                    base = wv
                    m0 = m - NQL - NKVL
                qw = NKC * HD // 4
                for q4 in range(4):
                    c0 = m0 * NKC * HD + q4 * qw
                    # split the weight stream across two rings so the
                    # Q-head loops never catch up with a clogged DMA queue;
                    # hf0 Q-heads use sync (not scalar) so their descriptor
                    # issues don't sit ahead of the attention exps in the
                    # ACT queue
                    if q4 % 2 == 0:
                        eng = nc.gpsimd
                    elif hf == 0 and m < NQL:
                        eng = nc.sync
                    else:
                        eng = nc.scalar
                    eng.dma_start(out=wsl[:, q4 * qw:(q4 + 1) * qw],
                                  in_=base[:, c0:c0 + qw])
                wsls[(hf, m)] = wsl
                return wsl

            # ---- xT loads: per (hf, kc) one tile [128, 1024].  The first
            # few kc tiles are issued before everything else so the 8-core
            # HBM rush at startup serves the critical path first ----
            xts = {}

            def load_xt(hf, kcs=None):
                for kc in (range(NKC) if kcs is None else kcs):
                    if (hf, kc) in xts:
                        continue
                    t = sb.tile([128, HW], BF16, name=f"xt{hf}_{kc}",
                                tag="xt", bufs=32)
                    if kc % 2 == 0:
                        eng = nc.sync
                    elif hf == 0:
                        eng = nc.scalar
                    else:
                        eng = nc.gpsimd
                    eng.dma_start(
                        out=t[:],
                        in_=xt_d[kc * 128:(kc + 1) * 128, hf * HW:(hf + 1) * HW])
                    xts[(hf, kc)] = t

            load_xt(0, range(6))

            # K/V head weights for the grouped pass: interleave the four
            # column-quarters across heads so every head's first quarter
            # lands before any head's second quarter
            for m0_ in range(NQL, NQL + 2 * NKVL):
                wsls[(0, m0_)] = sb.tile([128, NKC * HD], BF16,
                                         name=f"w0_{m0_}", tag="wsl", bufs=4)
            qw0 = NKC * HD // 4
            for q4 in range(4):
                for m0_ in range(NQL, NQL + 2 * NKVL):
                    if m0_ < NQL + NKVL:
                        base0 = wk
                        mb0 = m0_ - NQL
                    else:
                        base0 = wv
                        mb0 = m0_ - NQL - NKVL
                    c0 = mb0 * NKC * HD + q4 * qw0
                    eng0 = nc.scalar if q4 % 2 == 1 else nc.gpsimd
                    eng0.dma_start(
                        out=wsls[(0, m0_)][:, q4 * qw0:(q4 + 1) * qw0],
                        in_=base0[:, c0:c0 + qw0])

            # ---- constants ----
            ident = const.tile([128, 128], BF16, name="ident")
            nc.gpsimd.memset(ident[:], 0.0)
            nc.gpsimd.affine_select(
                out=ident[:], in_=ident[:],
                compare_op=mybir.AluOpType.not_equal, fill=1.0,
                base=0, pattern=[[-1, 128]], channel_multiplier=1,
            )
            ones = const.tile([128, 128], BF16, name="ones")
            nc.gpsimd.memset(ones[:], 1.0)
            # causal mask as a bf16 matmul operand: applied to the score
            # PSUM via an accumulating PE matmul (keeps the mask off DVE)
            maskbf = const.tile([128, 128], BF16, name="maskbf")
            nc.gpsimd.memset(maskbf[:], 0.0)
            nc.gpsimd.affine_select(
                out=maskbf[:], in_=maskbf[:],
                compare_op=mybir.AluOpType.is_ge, fill=NEG,
                base=0, pattern=[[1, 128]], channel_multiplier=-1,
            )
            # RoPE tables ride the sync ring (empty at start) so the PE
            # warmup matmuls aren't stuck behind the weight-quarter stream
            cos2 = const.tile([128, S], BF16, name="cos2")
            sin2n = const.tile([128, S], BF16, name="sin2n")
            for hh in range(2):
                hsl = slice(hh * (S // 2), (hh + 1) * (S // 2))
                nc.sync.dma_start(out=cos2[:, hsl], in_=cosp[:, hsl])
                nc.sync.dma_start(out=sin2n[:, hsl], in_=sinp[:, hsl])

            # PE p-state warmup: dummy matmuls ramp the array to full clock
            # (touch every PSUM tag: 2 proj + 2 sc pair-tiles + 2 o)
            warm_dsts = []
            for wi in range(2):
                warm_dsts.append(ps.tile([128, SCW], F32, name=f"pwp{wi}",
                                         tag="proj", bufs=2))
            for wi in range(2):
                wsc = ps.tile([128, 2 * SCW], F32, name=f"pws{wi}",
                              tag="sc", bufs=2)
                warm_dsts.append(wsc[:, 0:SCW])
                warm_dsts.append(wsc[:, SCW:2 * SCW])
            for wi in range(2):
                warm_dsts.append(ps.tile([128, SCW], F32, name=f"pwo{wi}",
                                         tag="o", bufs=2))
            ksb = big.tile([128, NKVL * S], BF16, name="ksb")
            vsb = big.tile([128, NTC * NKVL * HD], BF16, name="vsb")
            # attention outputs for the full sequence
            attnT = [big.tile([128, S], BF16, name=f"at{h}") for h in range(NQL)]

            # warmup copies scribble into attnT (overwritten later)
            for wi, pw in enumerate(warm_dsts):
                nc.tensor.matmul(pw, cos2[:, 0:128], cos2[:, 0:SCW],
                                 start=True, stop=True)
                nc.vector.tensor_copy(
                    out=attnT[wi // 4][:, (wi % 4) * SCW:(wi % 4 + 1) * SCW],
                    in_=pw)

            # remaining xT tiles (first few were front-loaded above)
            load_xt(0)

            def rope_or_v(hf, m, scq, pp, qtiles):
                sc = hf * 2 + scq
                ssl = slice(sc * SCW, (sc + 1) * SCW)
                qsl = slice(scq * SCW, (scq + 1) * SCW)
                if m < NQL + NKVL:
                    if m < NQL:
                        dst = qtiles[m][:, qsl]
                    else:
                        kv = m - NQL
                        dst = ksb[:, kv * S + sc * SCW:kv * S + (sc + 1) * SCW]
                    t1 = sb.tile([128, SCW], BF16, name=f"t1_{hf}_{m}_{scq}",
                                 tag="t1", bufs=2)
                    t2 = sb.tile([128, SCW], BF16, name=f"t2_{hf}_{m}_{scq}",
                                 tag="t2", bufs=2)
                    nc.vector.tensor_tensor(
                        out=t1[0:64, :], in0=pp[64:128, :],
                        in1=sin2n[0:64, ssl], op=mybir.AluOpType.mult)
                    nc.vector.tensor_tensor(
                        out=t1[64:128, :], in0=pp[0:64, :],
                        in1=sin2n[64:128, ssl], op=mybir.AluOpType.mult)
                    nc.vector.tensor_tensor(
                        out=t2[:], in0=pp[:], in1=cos2[:, ssl],
                        op=mybir.AluOpType.mult)
                    nc.vector.tensor_tensor(
                        out=dst, in0=t1[:], in1=t2[:],
                        op=mybir.AluOpType.add)
                else:
                    kv = m - NQL - NKVL
                    vts = sb.tile([128, SCW], BF16, name=f"vts{hf}_{kv}_{scq}",
                                  tag="vts", bufs=4)
                    nc.scalar.copy(out=vts[:], in_=pp)
                    transpose_v(hf, kv, scq, vts)

            def transpose_v(hf, kv, scq, vts):
                sc = hf * 2 + scq
                for j in range(SCW // 128):
                    pv = ps.tile([128, 128], BF16,
                                 name=f"pv{hf}_{kv}_{scq}_{j}",
                                 tag="sc", bufs=2)
                    nc.tensor.transpose(
                        pv[:], vts[:, j * 128:(j + 1) * 128], ident[:])
                    slot = (sc * 4 + j) * NKVL + kv
                    nc.scalar.copy(
                        out=vsb[:, slot * HD:(slot + 1) * HD], in_=pv[:])

            # ---- wo output projection as a resumable generator ----------
            oblk = NQL * SCW
            wots = {}

            def load_wot(li, dc):
                wot = sb.tile([128, NQL * SCW], BF16, name=f"wot{li}_{dc}",
                              tag="wot", bufs=2)
                qw = oblk // 4
                for q4 in range(4):
                    nc.gpsimd.dma_start(
                        out=wot[:, q4 * qw:(q4 + 1) * qw],
                        in_=wo[:, dc * oblk + q4 * qw:dc * oblk + (q4 + 1) * qw])
                wots[(li, dc)] = wot

            # after the attention phases complete, the sc/o PSUM banks are
            # idle: drain-mode wo accumulators cycle through all three tags
            # (6-deep pipeline) so dc-boundary os-copy chains never stall PE
            wo_drain = [False]

            def wo_gen(li, ssub_lo, ssub_hi, last_pass):
                """Yields once per wo matmul; boundary ops emitted inline."""
                load_wot(li, 0)
                load_wot(li, 1)
                cnt = 0
                for dc in range(D // SCW):
                    wot = wots[(li, dc)]
                    if dc + 2 < D // SCW:
                        load_wot(li, dc + 2)
                    for ssub in range(ssub_lo, ssub_hi):
                        s0 = ssub * 128
                        tg = ("proj", "sc", "o")[cnt % 3] if wo_drain[0] else "proj"
                        pd = ps.tile([128, SCW], F32, name=f"pd{li}_{dc}_{ssub}",
                                     tag=tg, bufs=2)
                        for kc8 in range(NQL):
                            nc.tensor.matmul(
                                pd[:],
                                attnT[kc8][:, s0:s0 + 128],
                                wot[:, kc8 * SCW:(kc8 + 1) * SCW],
                                start=(kc8 == 0), stop=(kc8 == NQL - 1))
                            yield
                        os_ = sb.tile([128, SCW], BF16, name=f"os{li}_{dc}_{ssub}",
                                      tag="os", bufs=4)
                        if wo_drain[0] and cnt % 2 == 0:
                            nc.scalar.copy(out=os_[:], in_=pd[:])
                        else:
                            nc.vector.tensor_copy(out=os_[:], in_=pd[:])
                        if last_pass and dc == D // SCW - 1:
                            oeng = (nc.sync, nc.gpsimd, nc.scalar)[ssub % 3]
                        else:
                            oeng = nc.sync if ssub % 2 == 0 else nc.gpsimd
                        oeng.dma_start(
                            out=out[s0:s0 + 128, dc * SCW:(dc + 1) * SCW],
                            in_=os_[:])
                        cnt += 1

            # ---- attention for one half: pair-pipelined ------------------
            def attention_half(hf, qtiles, filler, fill_rate):
                pairs = []
                for h in range(NQL):
                    for scq in range(2):
                        sc = hf * 2 + scq
                        for k in range(2 * sc + 2):
                            pairs.append((h, scq, k))
                n = len(pairs)
                psps = {}    # (h, scq, k) -> score pair tile [128, 1024] PSUM
                pts = {}     # (h, scq, k) -> pt pair tile [128, 1024]
                tmps = {}    # (h, scq, k) -> prereduced tile (full pairs)
                pos = {}
                prs = {}
                fill_acc = [0.0]

                def geom(sc, tcx):
                    j = tcx - 4 * sc
                    off = j * 128 if j > 0 else 0
                    return j, off, SCW - off

                def stage_a(h, scq, k):
                    # scores for both tiles of the pair + diagonal masks
                    sc = hf * 2 + scq
                    kv = h // (NQL // NKVL)
                    psp = ps.tile([128, 2 * SCW], F32, name=f"psp{sc}_{h}_{k}",
                                  tag="sc", bufs=2)
                    psps[(h, scq, k)] = psp
                    for half in range(2):
                        tcx = 2 * k + half
                        j, off, w = geom(sc, tcx)
                        qs0 = scq * SCW + off
                        diag = j >= 0
                        nc.tensor.matmul(
                            psp[:, half * SCW:half * SCW + w],
                            ksb[:, kv * S + tcx * 128:kv * S + (tcx + 1) * 128],
                            qtiles[h][:, qs0:qs0 + w],
                            start=True, stop=not diag,
                        )
                        if diag:
                            # accumulate the causal mask on PE (keeps DVE
                            # out of the attention critical path)
                            nc.tensor.matmul(
                                psp[:, half * SCW:half * SCW + 128],
                                ident[:], maskbf[:],
                                start=False, stop=True)

                def stage_b(h, scq, k):
                    # one exp ACTIVATE per pair, then DVE pre-reduction
                    sc = hf * 2 + scq
                    psp = psps.pop((h, scq, k))
                    _, _, w0 = geom(sc, 2 * k)
                    _, _, w1 = geom(sc, 2 * k + 1)
                    pt = sb.tile([128, 2 * SCW], BF16, name=f"pt{sc}_{h}_{k}",
                                 tag="pt", bufs=2)
                    if MERGE_EXP and w0 == SCW:
                        # contiguous 2D span over both halves (only when the
                        # first half is fully written - no junk-gap reads)
                        nc.scalar.activation(
                            pt[:, 0:SCW + w1], psp[:, 0:SCW + w1],
                            mybir.ActivationFunctionType.Exp, scale=SCALE)
                    else:
                        nc.scalar.activation(
                            pt[:, 0:w0], psp[:, 0:w0],
                            mybir.ActivationFunctionType.Exp, scale=SCALE)
                        nc.scalar.activation(
                            pt[:, SCW:SCW + w1], psp[:, SCW:SCW + w1],
                            mybir.ActivationFunctionType.Exp, scale=SCALE)
                    pts[(h, scq, k)] = pt
                    if PREREDUCE and w0 == SCW and w1 == SCW:
                        tmp = sb.tile([128, SCW], BF16, name=f"tm{sc}_{h}_{k}",
                                      tag="tmp", bufs=2)
                        nc.vector.tensor_tensor(
                            out=tmp[:], in0=pt[:, 0:SCW], in1=pt[:, SCW:2 * SCW],
                            op=mybir.AluOpType.add)
                        tmps[(h, scq, k)] = tmp

                def stage_c(h, scq, k):
                    # PV matmuls for both tiles of the pair
                    sc = hf * 2 + scq
                    ntc = 4 * sc + 4
                    kv = h // (NQL // NKVL)
                    if k == 0:
                        # po and pr alternate through the same 2-buf tag
                        pos[(h, scq)] = ps.tile([128, SCW], F32,
                                                name=f"po{sc}_{h}", tag="o",
                                                bufs=2)
                        prs[(h, scq)] = ps.tile([128, SCW], F32,
                                                name=f"pr{sc}_{h}", tag="o",
                                                bufs=2)
                    po = pos[(h, scq)]
                    pt = pts[(h, scq, k)]
                    for half in range(2):
                        tcx = 2 * k + half
                        j, off, w = geom(sc, tcx)
                        slot = tcx * NKVL + kv
                        nc.tensor.matmul(
                            po[:, off:], vsb[:, slot * HD:(slot + 1) * HD],
                            pt[:, half * SCW:half * SCW + w],
                            start=(tcx == 0), stop=(tcx == ntc - 1))

                def stage_d(h, scq, k):
                    # rowsum matmul(s) for the pair; normalize at group end
                    sc = hf * 2 + scq
                    npr = 2 * sc + 2
                    pr = prs[(h, scq)]
                    pt = pts.pop((h, scq, k))
                    tmp = tmps.pop((h, scq, k), None)
                    if tmp is not None:
                        nc.tensor.matmul(
                            pr[:], ones[:], tmp[:],
                            start=(k == 0), stop=False)
                    else:
                        for half in range(2):
                            tcx = 2 * k + half
                            j, off, w = geom(sc, tcx)
                            nc.tensor.matmul(
                                pr[:, off:], ones[:], pt[:, half * SCW:half * SCW + w],
                                start=(k == 0 and half == 0),
                                stop=(k == npr - 1 and half == 1))
                    if k == npr - 1:
                        po = pos.pop((h, scq))
                        prs.pop((h, scq))
                        rec = sb.tile([128, SCW], F32, name=f"rec{sc}_{h}",
                                      tag="rec", bufs=1)
                        nc.vector.reciprocal_approx_fast(out=rec[:], in_=pr[:])
                        if DEBUG_DUMP and h == 0:
                            pcp = sb.tile([128, SCW], F32, name=f"pcp{sc}",
                                          tag="dbgp", bufs=1)
                            nc.vector.tensor_copy(out=pcp[:], in_=po[:])
                            nc.sync.dma_start(
                                out=dbg["dbg_po"][:, sc * SCW:(sc + 1) * SCW],
                                in_=pcp[:])
                            nc.sync.dma_start(
                                out=dbg["dbg_rec"][:, sc * SCW:(sc + 1) * SCW],
                                in_=rec[:])
                        nc.vector.tensor_tensor(
                            out=attnT[h][:, sc * SCW:(sc + 1) * SCW],
                            in0=po[:], in1=rec[:],
                            op=mybir.AluOpType.mult)

                for i in range(n + 2):
                    if i < n:
                        stage_a(*pairs[i])
                    if 0 <= i - 1 < n:
                        stage_b(*pairs[i - 1])
                    if filler is not None and i >= 2:
                        fill_acc[0] += fill_rate
                        while fill_acc[0] >= 1.0:
                            if next(filler, None) is None:
                                filler = None
                                break
                            fill_acc[0] -= 1.0
                    if 0 <= i - 2 < n:
                        stage_c(*pairs[i - 2])
                        stage_d(*pairs[i - 2])

            # ================= main schedule =================
            wo0 = None
            for hf in range(2):
                qtiles = [None] * NQL
                if hf == 0:
                    # grouped pass over the 4 K/V heads x both s-chunks
                    grp = list(range(NQL, NM))
                    wsl4 = {m: wsls[(0, m)] for m in grp}
                    accums = []
                    for wi in range(2):
                        accums.append(ps.tile([128, SCW], F32, name=f"gp{wi}",
                                              tag="proj", bufs=2))
                    for wi in range(2):
                        gsc = ps.tile([128, 2 * SCW], F32, name=f"gsc{wi}",
                                      tag="sc", bufs=2)
                        accums.append(gsc[:, 0:SCW])
                        accums.append(gsc[:, SCW:2 * SCW])
                    for wi in range(2):
                        accums.append(ps.tile([128, SCW], F32, name=f"go{wi}",
                                              tag="o", bufs=2))
                    pp8 = {}
                    for gi, (m, scq) in enumerate(
                            (m, s) for m in grp for s in range(2)):
                        pp8[(m, scq)] = accums[gi]
                    for kc in range(NKC):
                        for m in grp:
                            for scq in range(2):
                                nc.tensor.matmul(
                                    pp8[(m, scq)],
                                    wsl4[m][:, kc * 128:(kc + 1) * 128],
                                    xts[(0, kc)][:, scq * SCW:(scq + 1) * SCW],
                                    start=(kc == 0), stop=(kc == NKC - 1),
                                )
                    # K ropes first, then free ALL V accumulators via ACT
                    # copies before any pv transpose reuses their sc slots
                    for m in (NQL, NQL + 1):
                        for scq in range(2):
                            rope_or_v(0, m, scq, pp8[(m, scq)], qtiles)
                    gvts = {}
                    for m in (NQL + NKVL, NQL + NKVL + 1):
                        for scq in range(2):
                            kv = m - NQL - NKVL
                            vts = sb.tile([128, SCW], BF16,
                                          name=f"gv{kv}_{scq}", tag="vts",
                                          bufs=4)
                            nc.scalar.copy(out=vts[:], in_=pp8[(m, scq)])
                            gvts[(kv, scq)] = vts
                    for (kv, scq), vts in gvts.items():
                        transpose_v(0, kv, scq, vts)
                    morder = list(range(NQL))
                else:
                    # K heads already projected via the hf0-attention
                    # filler; V heads then q heads remain
                    morder = [NQL + NKVL, NQL + NKVL + 1] + list(range(NQL))
                for m in morder:
                    wsl = load_wsl(hf, m)
                    if m < NQL:
                        qt = sb.tile([128, HW], BF16, name=f"q{hf}_{m}",
                                     tag=f"q{m}", bufs=1)
                        qtiles[m] = qt
                    for scq in range(2):
                        qsl = slice(scq * SCW, (scq + 1) * SCW)
                        pp = ps.tile([128, SCW], F32, name=f"pp{hf}_{m}_{scq}",
                                     tag="proj", bufs=2)
                        for kc in range(NKC):
                            nc.tensor.matmul(
                                pp[:], wsl[:, kc * 128:(kc + 1) * 128],
                                xts[(hf, kc)][:, qsl],
                                start=(kc == 0), stop=(kc == NKC - 1),
                            )
                        rope_or_v(hf, m, scq, pp, qtiles)

                if hf == 0:
                    # hf1 K-head weights first so the attention filler's
                    # first matmuls aren't stuck behind the x DMA queue
                    load_wsl(1, NQL)
                    load_wsl(1, NQL + 1)
                    load_xt(1)
                    if DEBUG_DUMP:
                        nc.sync.dma_start(out=dbg["dbg_q0"][:, :], in_=qtiles[0][:])

                    def kv1_gen():
                        # hf1 K-head projections, injected into hf0 attn
                        for m in (NQL, NQL + 1):
                            wsl1 = wsls[(1, m)]
                            for scq in range(2):
                                pp1 = ps.tile([128, SCW], F32,
                                              name=f"kp{m}_{scq}",
                                              tag="proj", bufs=2)
                                for kc in range(NKC):
                                    nc.tensor.matmul(
                                        pp1[:],
                                        wsl1[:, kc * 128:(kc + 1) * 128],
                                        xts[(1, kc)][:, scq * SCW:(scq + 1) * SCW],
                                        start=(kc == 0), stop=(kc == NKC - 1),
                                    )
                                    yield
                                rope_or_v(1, m, scq, pp1, qtiles)

                    kv1 = kv1_gen()
                    attention_half(0, qtiles, kv1, 2.0)
                    for _ in kv1:
                        pass
                else:
                    wo0 = wo_gen(0, 0, S // 256, last_pass=False)
                    attention_half(1, qtiles, wo0, 1.5)

            # drain remaining wo work: rest of first half, then second half
            wo_drain[0] = True
            for _ in wo0:
                pass
            for _ in wo_gen(1, S // 256, S // 128, last_pass=True):
                pass
            if DEBUG_DUMP:
                nc.sync.dma_start(out=dbg["dbg_ksb"][:, :], in_=ksb[:])
                nc.sync.dma_start(out=dbg["dbg_vsb"][:, :], in_=vsb[:])
                nc.sync.dma_start(out=dbg["dbg_at0"][:, :], in_=attnT[0][:])
    nc.finalize()
    return nc


_NC_CACHE = None


def _get_graph():
    global _NC_CACHE
    if _NC_CACHE is None:
        _NC_CACHE = _build()
    return _NC_CACHE


_PERM = np.concatenate([np.arange(0, HD, 2), np.arange(1, HD, 2)])


def _tile_w(w):
    """[D, M*HD] -> [128, m-major kc-major 128cols] contiguous tiling (bf16)."""
    d, mc = w.shape
    nm = mc // HD
    t = w.reshape(NKC, 128, nm, HD).transpose(1, 2, 0, 3)
    return np.ascontiguousarray(t.reshape(128, nm * NKC * HD).astype(NPBF))


def _tile_wo(w):
    """[NQL*HD, D] -> [128, dc-major kc-major 512cols] (bf16)."""
    t = w.reshape(NQL, 128, D // SCW, SCW).transpose(1, 2, 0, 3)
    return np.ascontiguousarray(
        t.reshape(128, (D // SCW) * NQL * SCW).astype(NPBF))


def _shard_inputs(x, freqs_cos, freqs_sin, wq, wk, wv, wo):
    """Build the 8 per-core input maps (pure numpy prep, nothing on-device)."""
    x = np.asarray(x, dtype=np.float32)
    wq = np.asarray(wq, dtype=np.float32)
    wk = np.asarray(wk, dtype=np.float32)
    wv = np.asarray(wv, dtype=np.float32)
    wo = np.asarray(wo, dtype=np.float32)
    cos = np.asarray(freqs_cos, dtype=np.float32)
    sin = np.asarray(freqs_sin, dtype=np.float32)

    # RoPE tables in on-chip [128, S] layout
    cos2 = np.empty((128, S), np.float32)
    sin2n = np.empty((128, S), np.float32)
    cos2[0:64] = cos.T
    cos2[64:128] = cos.T
    sin2n[0:64] = -sin.T
    sin2n[64:128] = sin.T
    cos2 = cos2.astype(NPBF)
    sin2n = sin2n.astype(NPBF)

    wq4 = wq.reshape(D, NH, HD)
    wk4 = wk.reshape(D, NKV, HD)
    wv4 = wv.reshape(D, NKV, HD)
    wo4 = wo.reshape(NH, HD, D)

    # x transposed + bf16 per batch
    xts = [np.ascontiguousarray(x[b].T.astype(NPBF)) for b in range(B)]

    in_maps = []
    for c in range(NCORES):
        b, g = divmod(c, TPG)
        qh = slice(g * NQL, (g + 1) * NQL)
        kvh = slice(g * NKVL, (g + 1) * NKVL)
        m = {
            "xt": xts[b],
            "wq": _tile_w(wq4[:, qh, :][:, :, _PERM].reshape(D, NQL * HD)),
            "wk": _tile_w(wk4[:, kvh, :][:, :, _PERM].reshape(D, NKVL * HD)),
            "wv": _tile_w(wv4[:, kvh, :].reshape(D, NKVL * HD)),
            "wo": _tile_wo(wo4[qh].reshape(NQL * HD, D)),
            "cosp": cos2,
            "sinp": sin2n,
        }
        in_maps.append(m)
    return in_maps


def kernel(x, start_pos, freqs_cos, freqs_sin, mask, wq, wk, wv, wo,
           cache_k, cache_v):
    x = np.asarray(x)
    in_maps = _shard_inputs(x, freqs_cos, freqs_sin, wq, wk, wv, wo)
    nc = _get_graph()
    res = run_bass_kernel_spmd(nc, in_maps, core_ids=list(range(NCORES)))
    out = np.zeros((B, S, D), dtype=np.float32)
    for b in range(B):
        acc = np.asarray(res.results[b * TPG]["out"]).astype(np.float32)
        for g in range(1, TPG):
            acc += np.asarray(res.results[b * TPG + g]["out"]).astype(np.float32)
        out[b] = acc
    return out


# revision 43
# speedup vs baseline: 1.1998x; 1.0089x over previous
"""Distributed Trainium2 Bass kernel for GQA attention prefill.

Problem: B=2, S=2048, D=4096, 32 q heads, 8 kv heads, head_dim=128, RoPE,
causal mask, start_pos=0.

Sharding (8 cores): DP2 over batch x TP4 over heads.  Core c = b*4 + g gets
batch b, q-heads 8g..8g+7, kv-heads 2g..2g+1, wo rows for those q-heads.
Each core computes a partial [S, D] output (bf16); the host sums the 4
partials per batch (the row-parallel wo unshard).

All layout work happens on the host: x arrives pre-transposed and pre-cast
to bf16, weights arrive bf16 pre-tiled, RoPE tables arrive precomputed in
their on-chip [128, S] layout.

On-core dataflow per half (1024 query positions):
  load xT bf16 tiles [128, 1024]; QKV projection (bf16 matmuls); RoPE on
  the projection PSUM.  Attention runs a pair-pipelined software pipeline:
  each PAIR of 128-key score tiles lands in one 2-bank PSUM pool tile, exp
  runs one ACTIVATE per pair (halving ACT instruction overhead), a DVE
  pre-reduction sums each full pair of P tiles so the softmax denominator
  needs one ones-matmul per pair instead of two (cuts PE rowsum cycles),
  and the causal mask is accumulated by an extra PE matmul (ident x mask)
  instead of a DVE add, keeping DVE off the critical path.  PE bubbles in
  the ACT/DVE-paced attention phases are filled with injected matmuls:
  the next half's K-head projections during the first half's attention,
  and the wo output projection for s<1024 during the second half's.  The
  remaining wo work runs as a tail (wo streamed once per s-half), with
  PSUM->SBUF copies on ACT and output DMAs alternating rings.
"""

import math

import numpy as np
import ml_dtypes

import concourse.bass as bass  # noqa: F401  (bass types via bacc)
import concourse.mybir as mybir
from concourse import bacc
from concourse.bass_utils import run_bass_kernel_spmd
from concourse.tile import TileContext

F32 = mybir.dt.float32
BF16 = mybir.dt.bfloat16
NPBF = ml_dtypes.bfloat16

B, S, D = 2, 2048, 4096
NH, NKV, HD = 32, 8, 128
NCORES = 8
TPG = 4                  # tensor-parallel groups
NQL = NH // TPG          # 8 local q heads
NKVL = NKV // TPG        # 2 local kv heads
SCW = 512                # s-chunk width
HW = S // 2              # half width (1024)
NKC = D // 128           # 32 contraction chunks for projections
NTC = S // 128           # 16 T-chunks (key positions)
SCALE = 1.0 / math.sqrt(HD)
NEG = -1e9
MERGE_EXP = True      # one ACTIVATE per adjacent-slot score pair
PREREDUCE = True      # DVE pair pre-reduction + single rowsum matmul
DEBUG_DUMP = False    # add debug DRAM outputs (sim bisection only)


def _build():
    nc = bacc.Bacc("TRN2", target_bir_lowering=False, debug=False,
                   num_devices=NCORES)
    # x pre-transposed + pre-cast: [D, S] bf16
    xt_d = nc.declare_dram_parameter("xt", [D, S], BF16, isOutput=False)
    # weights arrive pre-tiled bf16: [128, m-major kc-major cols]
    wq = nc.declare_dram_parameter("wq", [128, NQL * NKC * HD], BF16, isOutput=False)
    wk = nc.declare_dram_parameter("wk", [128, NKVL * NKC * HD], BF16, isOutput=False)
    wv = nc.declare_dram_parameter("wv", [128, NKVL * NKC * HD], BF16, isOutput=False)
    wo = nc.declare_dram_parameter("wo", [128, (D // SCW) * NQL * SCW], BF16, isOutput=False)
    # RoPE tables in on-chip layout [128, S]
    cosp = nc.declare_dram_parameter("cosp", [128, S], BF16, isOutput=False)
    sinp = nc.declare_dram_parameter("sinp", [128, S], BF16, isOutput=False)
    out = nc.declare_dram_parameter("out", [S, D], BF16, isOutput=True)
    if DEBUG_DUMP:
        dbg = {
            "dbg_ksb": nc.declare_dram_parameter("dbg_ksb", [128, NKVL * S], BF16, isOutput=True),
            "dbg_vsb": nc.declare_dram_parameter("dbg_vsb", [128, NTC * NKVL * HD], BF16, isOutput=True),
            "dbg_q0": nc.declare_dram_parameter("dbg_q0", [128, HW], BF16, isOutput=True),
            "dbg_at0": nc.declare_dram_parameter("dbg_at0", [128, S], BF16, isOutput=True),
            "dbg_rec": nc.declare_dram_parameter("dbg_rec", [128, 4 * SCW], F32, isOutput=True),
            "dbg_po": nc.declare_dram_parameter("dbg_po", [128, 4 * SCW], F32, isOutput=True),
        }

    NM = NQL + 2 * NKVL

    with TileContext(nc) as tc:
        with (
            tc.tile_pool(name="const", bufs=1) as const,
            tc.tile_pool(name="big", bufs=1) as big,
            tc.tile_pool(name="sb", bufs=3) as sb,
            tc.tile_pool(name="ps", bufs=1, space="PSUM") as ps,
        ):
            # ---- weight-slice loader ----
            wsls = {}

            def load_wsl(hf, m):
                if (hf, m) in wsls:
                    return wsls[(hf, m)]
                wsl = sb.tile([128, NKC * HD], BF16, name=f"w{hf}_{m}",
                              tag="wsl", bufs=4)
                if m < NQL:
                    base = wq
                    m0 = m
                elif m < NQL + NKVL:
                    base = wk
                    m0 = m - NQL
                else:
                    base = wv
                    m0 = m - NQL - NKVL
                qw = NKC * HD // 4
                for q4 in range(4):
                    c0 = m0 * NKC * HD + q4 * qw
                    # split the weight stream across two rings so the
                    # Q-head loops never catch up with a clogged DMA queue;
                    # hf0 Q-heads use sync (not scalar) so their descriptor
                    # issues don't sit ahead of the attention exps in the
                    # ACT queue, and the pre-attention hf1 KV loads stay on
                    # gpsimd entirely for the same reason (their slot WAR
                    # releases mid-attention and would head-of-line block
                    # the first exps)
                    if q4 % 2 == 0:
                        eng = nc.gpsimd
                    elif hf == 0 and m < NQL:
                        eng = nc.sync
                    elif hf == 1 and m >= NQL:
                        eng = nc.gpsimd
                    else:
                        eng = nc.scalar
                    eng.dma_start(out=wsl[:, q4 * qw:(q4 + 1) * qw],
                                  in_=base[:, c0:c0 + qw])
                wsls[(hf, m)] = wsl
                return wsl

            # ---- xT loads: per (hf, kc) one tile [128, 1024].  The first
            # few kc tiles are issued before everything else so the 8-core
            # HBM rush at startup serves the critical path first ----
            xts = {}

            def load_xt(hf, kcs=None):
                for kc in (range(NKC) if kcs is None else kcs):
                    if (hf, kc) in xts:
                        continue
                    t = sb.tile([128, HW], BF16, name=f"xt{hf}_{kc}",
                                tag="xt", bufs=32)
                    if kc % 2 == 0:
                        eng = nc.sync
                    elif hf == 0:
                        eng = nc.scalar
                    else:
                        eng = nc.gpsimd
                    eng.dma_start(
                        out=t[:],
                        in_=xt_d[kc * 128:(kc + 1) * 128, hf * HW:(hf + 1) * HW])
                    xts[(hf, kc)] = t

            load_xt(0, range(6))

            # K/V head weights for the grouped pass: interleave the four
            # column-quarters across heads so every head's first quarter
            # lands before any head's second quarter
            for m0_ in range(NQL, NQL + 2 * NKVL):
                wsls[(0, m0_)] = sb.tile([128, NKC * HD], BF16,
                                         name=f"w0_{m0_}", tag="wsl", bufs=4)
            qw0 = NKC * HD // 4
            for q4 in range(4):
                for m0_ in range(NQL, NQL + 2 * NKVL):
                    if m0_ < NQL + NKVL:
                        base0 = wk
                        mb0 = m0_ - NQL
                    else:
                        base0 = wv
                        mb0 = m0_ - NQL - NKVL
                    c0 = mb0 * NKC * HD + q4 * qw0
                    eng0 = nc.scalar if q4 % 2 == 1 else nc.gpsimd
                    eng0.dma_start(
                        out=wsls[(0, m0_)][:, q4 * qw0:(q4 + 1) * qw0],
                        in_=base0[:, c0:c0 + qw0])

            # ---- constants ----
            ident = const.tile([128, 128], BF16, name="ident")
            nc.gpsimd.memset(ident[:], 0.0)
            nc.gpsimd.affine_select(
                out=ident[:], in_=ident[:],
                compare_op=mybir.AluOpType.not_equal, fill=1.0,
                base=0, pattern=[[-1, 128]], channel_multiplier=1,
            )
            ones = const.tile([128, 128], BF16, name="ones")
            nc.gpsimd.memset(ones[:], 1.0)
            # causal mask as a bf16 matmul operand: applied to the score
            # PSUM via an accumulating PE matmul (keeps the mask off DVE)
            maskbf = const.tile([128, 128], BF16, name="maskbf")
            nc.gpsimd.memset(maskbf[:], 0.0)
            nc.gpsimd.affine_select(
                out=maskbf[:], in_=maskbf[:],
                compare_op=mybir.AluOpType.is_ge, fill=NEG,
                base=0, pattern=[[1, 128]], channel_multiplier=-1,
            )
            # RoPE tables ride the sync ring (empty at start) so the PE
            # warmup matmuls aren't stuck behind the weight-quarter stream
            cos2 = const.tile([128, S], BF16, name="cos2")
            sin2n = const.tile([128, S], BF16, name="sin2n")
            for hh in range(2):
                hsl = slice(hh * (S // 2), (hh + 1) * (S // 2))
                nc.sync.dma_start(out=cos2[:, hsl], in_=cosp[:, hsl])
                nc.sync.dma_start(out=sin2n[:, hsl], in_=sinp[:, hsl])

            # PE p-state warmup: dummy matmuls ramp the array to full clock
            # (touch every PSUM tag: 2 proj + 2 sc pair-tiles + 2 o)
            warm_dsts = []
            for wi in range(2):
                warm_dsts.append(ps.tile([128, SCW], F32, name=f"pwp{wi}",
                                         tag="proj", bufs=2))
            for wi in range(2):
                wsc = ps.tile([128, 2 * SCW], F32, name=f"pws{wi}",
                              tag="sc", bufs=2)
                warm_dsts.append(wsc[:, 0:SCW])
                warm_dsts.append(wsc[:, SCW:2 * SCW])
            for wi in range(2):
                warm_dsts.append(ps.tile([128, SCW], F32, name=f"pwo{wi}",
                                         tag="o", bufs=2))
            ksb = big.tile([128, NKVL * S], BF16, name="ksb")
            vsb = big.tile([128, NTC * NKVL * HD], BF16, name="vsb")
            # attention outputs for the full sequence
            attnT = [big.tile([128, S], BF16, name=f"at{h}") for h in range(NQL)]

            # warmup copies scribble into attnT (overwritten later)
            for wi, pw in enumerate(warm_dsts):
                nc.tensor.matmul(pw, cos2[:, 0:128], cos2[:, 0:SCW],
                                 start=True, stop=True)
                nc.vector.tensor_copy(
                    out=attnT[wi // 4][:, (wi % 4) * SCW:(wi % 4 + 1) * SCW],
                    in_=pw)

            # remaining xT tiles (first few were front-loaded above)
            load_xt(0)

            def rope_or_v(hf, m, scq, pp, qtiles):
                sc = hf * 2 + scq
                ssl = slice(sc * SCW, (sc + 1) * SCW)
                qsl = slice(scq * SCW, (scq + 1) * SCW)
                if m < NQL + NKVL:
                    if m < NQL:
                        dst = qtiles[m][:, qsl]
                    else:
                        kv = m - NQL
                        dst = ksb[:, kv * S + sc * SCW:kv * S + (sc + 1) * SCW]
                    t1 = sb.tile([128, SCW], BF16, name=f"t1_{hf}_{m}_{scq}",
                                 tag="t1", bufs=2)
                    t2 = sb.tile([128, SCW], BF16, name=f"t2_{hf}_{m}_{scq}",
                                 tag="t2", bufs=2)
                    nc.vector.tensor_tensor(
                        out=t1[0:64, :], in0=pp[64:128, :],
                        in1=sin2n[0:64, ssl], op=mybir.AluOpType.mult)
                    nc.vector.tensor_tensor(
                        out=t1[64:128, :], in0=pp[0:64, :],
                        in1=sin2n[64:128, ssl], op=mybir.AluOpType.mult)
                    nc.vector.tensor_tensor(
                        out=t2[:], in0=pp[:], in1=cos2[:, ssl],
                        op=mybir.AluOpType.mult)
                    nc.vector.tensor_tensor(
                        out=dst, in0=t1[:], in1=t2[:],
                        op=mybir.AluOpType.add)
                else:
                    kv = m - NQL - NKVL
                    vts = sb.tile([128, SCW], BF16, name=f"vts{hf}_{kv}_{scq}",
                                  tag="vts", bufs=4)
                    nc.scalar.copy(out=vts[:], in_=pp)
                    transpose_v(hf, kv, scq, vts)

            def transpose_v(hf, kv, scq, vts):
                sc = hf * 2 + scq
                for j in range(SCW // 128):
                    pv = ps.tile([128, 128], BF16,
                                 name=f"pv{hf}_{kv}_{scq}_{j}",
                                 tag="sc", bufs=2)
                    nc.tensor.transpose(
                        pv[:], vts[:, j * 128:(j + 1) * 128], ident[:])
                    slot = (sc * 4 + j) * NKVL + kv
                    nc.scalar.copy(
                        out=vsb[:, slot * HD:(slot + 1) * HD], in_=pv[:])

            # ---- wo output projection as a resumable generator ----------
            oblk = NQL * SCW
            wots = {}

            def load_wot(li, dc):
                wot = sb.tile([128, NQL * SCW], BF16, name=f"wot{li}_{dc}",
                              tag="wot", bufs=2)
                qw = oblk // 4
                for q4 in range(4):
                    nc.gpsimd.dma_start(
                        out=wot[:, q4 * qw:(q4 + 1) * qw],
                        in_=wo[:, dc * oblk + q4 * qw:dc * oblk + (q4 + 1) * qw])
                wots[(li, dc)] = wot

            # after the attention phases complete, the sc/o PSUM banks are
            # idle: drain-mode wo accumulators cycle through all three tags
            # (6-deep pipeline) so dc-boundary os-copy chains never stall PE
            wo_drain = [False]

            def wo_gen(li, ssub_lo, ssub_hi, last_pass):
                """Yields once per wo matmul; boundary ops emitted inline."""
                load_wot(li, 0)
                load_wot(li, 1)
                cnt = 0
                for dc in range(D // SCW):
                    wot = wots[(li, dc)]
                    if dc + 2 < D // SCW:
                        load_wot(li, dc + 2)
                    for ssub in range(ssub_lo, ssub_hi):
                        s0 = ssub * 128
                        tg = ("proj", "sc", "o")[cnt % 3] if wo_drain[0] else "proj"
                        pd = ps.tile([128, SCW], F32, name=f"pd{li}_{dc}_{ssub}",
                                     tag=tg, bufs=2)
                        for kc8 in range(NQL):
                            nc.tensor.matmul(
                                pd[:],
                                attnT[kc8][:, s0:s0 + 128],
                                wot[:, kc8 * SCW:(kc8 + 1) * SCW],
                                start=(kc8 == 0), stop=(kc8 == NQL - 1))
                            yield
                        os_ = sb.tile([128, SCW], BF16, name=f"os{li}_{dc}_{ssub}",
                                      tag="os", bufs=4)
                        if wo_drain[0] and cnt % 2 == 0:
                            nc.scalar.copy(out=os_[:], in_=pd[:])
                        else:
                            nc.vector.tensor_copy(out=os_[:], in_=pd[:])
                        if last_pass and dc == D // SCW - 1:
                            oeng = (nc.sync, nc.gpsimd, nc.scalar)[ssub % 3]
                        else:
                            oeng = nc.sync if ssub % 2 == 0 else nc.gpsimd
                        oeng.dma_start(
                            out=out[s0:s0 + 128, dc * SCW:(dc + 1) * SCW],
                            in_=os_[:])
                        cnt += 1

            # ---- attention for one half: pair-pipelined ------------------
            def attention_half(hf, qtiles, filler, fill_rate):
                pairs = []
                for h in range(NQL):
                    for scq in range(2):
                        sc = hf * 2 + scq
                        for k in range(2 * sc + 2):
                            pairs.append((h, scq, k))
                n = len(pairs)
                psps = {}    # (h, scq, k) -> score pair tile [128, 1024] PSUM
                pts = {}     # (h, scq, k) -> pt pair tile [128, 1024]
                tmps = {}    # (h, scq, k) -> prereduced tile (full pairs)
                pos = {}
                prs = {}
                fill_acc = [0.0]

                def geom(sc, tcx):
                    j = tcx - 4 * sc
                    off = j * 128 if j > 0 else 0
                    return j, off, SCW - off

                def stage_a(h, scq, k):
                    # scores for both tiles of the pair + diagonal masks
                    sc = hf * 2 + scq
                    kv = h // (NQL // NKVL)
                    psp = ps.tile([128, 2 * SCW], F32, name=f"psp{sc}_{h}_{k}",
                                  tag="sc", bufs=2)
                    psps[(h, scq, k)] = psp
                    for half in range(2):
                        tcx = 2 * k + half
                        j, off, w = geom(sc, tcx)
                        qs0 = scq * SCW + off
                        diag = j >= 0
                        nc.tensor.matmul(
                            psp[:, half * SCW:half * SCW + w],
                            ksb[:, kv * S + tcx * 128:kv * S + (tcx + 1) * 128],
                            qtiles[h][:, qs0:qs0 + w],
                            start=True, stop=not diag,
                        )
                        if diag:
                            # accumulate the causal mask on PE (keeps DVE
                            # out of the attention critical path)
                            nc.tensor.matmul(
                                psp[:, half * SCW:half * SCW + 128],
                                ident[:], maskbf[:],
                                start=False, stop=True)

                def stage_b(h, scq, k):
                    # one exp ACTIVATE per pair, then DVE pre-reduction
                    sc = hf * 2 + scq
                    psp = psps.pop((h, scq, k))
                    _, _, w0 = geom(sc, 2 * k)
                    _, _, w1 = geom(sc, 2 * k + 1)
                    pt = sb.tile([128, 2 * SCW], BF16, name=f"pt{sc}_{h}_{k}",
                                 tag="pt", bufs=2)
                    if MERGE_EXP and w0 == SCW:
                        # contiguous 2D span over both halves (only when the
                        # first half is fully written - no junk-gap reads)
                        nc.scalar.activation(
                            pt[:, 0:SCW + w1], psp[:, 0:SCW + w1],
                            mybir.ActivationFunctionType.Exp, scale=SCALE)
                    else:
                        nc.scalar.activation(
                            pt[:, 0:w0], psp[:, 0:w0],
                            mybir.ActivationFunctionType.Exp, scale=SCALE)
                        nc.scalar.activation(
                            pt[:, SCW:SCW + w1], psp[:, SCW:SCW + w1],
                            mybir.ActivationFunctionType.Exp, scale=SCALE)
                    pts[(h, scq, k)] = pt
                    if PREREDUCE and w0 == SCW and w1 == SCW:
                        tmp = sb.tile([128, SCW], BF16, name=f"tm{sc}_{h}_{k}",
                                      tag="tmp", bufs=2)
                        nc.vector.tensor_tensor(
                            out=tmp[:], in0=pt[:, 0:SCW], in1=pt[:, SCW:2 * SCW],
                            op=mybir.AluOpType.add)
                        tmps[(h, scq, k)] = tmp

                def stage_c(h, scq, k):
                    # PV matmuls for both tiles of the pair
                    sc = hf * 2 + scq
                    ntc = 4 * sc + 4
                    kv = h // (NQL // NKVL)
                    if k == 0:
                        # po and pr alternate through the same 2-buf tag
                        pos[(h, scq)] = ps.tile([128, SCW], F32,
                                                name=f"po{sc}_{h}", tag="o",
                                                bufs=2)
                        prs[(h, scq)] = ps.tile([128, SCW], F32,
                                                name=f"pr{sc}_{h}", tag="o",
                                                bufs=2)
                    po = pos[(h, scq)]
                    pt = pts[(h, scq, k)]
                    for half in range(2):
                        tcx = 2 * k + half
                        j, off, w = geom(sc, tcx)
                        slot = tcx * NKVL + kv
                        nc.tensor.matmul(
                            po[:, off:], vsb[:, slot * HD:(slot + 1) * HD],
                            pt[:, half * SCW:half * SCW + w],
                            start=(tcx == 0), stop=(tcx == ntc - 1))

                def stage_d(h, scq, k):
                    # rowsum matmul(s) for the pair; normalize at group end
                    sc = hf * 2 + scq
                    npr = 2 * sc + 2
                    pr = prs[(h, scq)]
                    pt = pts.pop((h, scq, k))
                    tmp = tmps.pop((h, scq, k), None)
                    if tmp is not None:
                        nc.tensor.matmul(
                            pr[:], ones[:], tmp[:],
                            start=(k == 0), stop=False)
                    else:
                        for half in range(2):
                            tcx = 2 * k + half
                            j, off, w = geom(sc, tcx)
                            nc.tensor.matmul(
                                pr[:, off:], ones[:], pt[:, half * SCW:half * SCW + w],
                                start=(k == 0 and half == 0),
                                stop=(k == npr - 1 and half == 1))
                    if k == npr - 1:
                        po = pos.pop((h, scq))
                        prs.pop((h, scq))
                        rec = sb.tile([128, SCW], F32, name=f"rec{sc}_{h}",
                                      tag="rec", bufs=1)
                        nc.vector.reciprocal_approx_fast(out=rec[:], in_=pr[:])
                        if DEBUG_DUMP and h == 0:
                            pcp = sb.tile([128, SCW], F32, name=f"pcp{sc}",
                                          tag="dbgp", bufs=1)
                            nc.vector.tensor_copy(out=pcp[:], in_=po[:])
                            nc.sync.dma_start(
                                out=dbg["dbg_po"][:, sc * SCW:(sc + 1) * SCW],
                                in_=pcp[:])
                            nc.sync.dma_start(
                                out=dbg["dbg_rec"][:, sc * SCW:(sc + 1) * SCW],
                                in_=rec[:])
                        nc.vector.tensor_tensor(
                            out=attnT[h][:, sc * SCW:(sc + 1) * SCW],
                            in0=po[:], in1=rec[:],
                            op=mybir.AluOpType.mult)

                for i in range(n + 2):
                    if i < n:
                        stage_a(*pairs[i])
                    if 0 <= i - 1 < n:
                        stage_b(*pairs[i - 1])
                    if filler is not None and i >= 2:
                        fill_acc[0] += fill_rate
                        while fill_acc[0] >= 1.0:
                            if next(filler, None) is None:
                                filler = None
                                break
                            fill_acc[0] -= 1.0
                    if 0 <= i - 2 < n:
                        stage_c(*pairs[i - 2])
                        stage_d(*pairs[i - 2])

            # ================= main schedule =================
            wo0 = None
            for hf in range(2):
                qtiles = [None] * NQL
                if hf == 0:
                    # grouped pass over the 4 K/V heads x both s-chunks
                    grp = list(range(NQL, NM))
                    wsl4 = {m: wsls[(0, m)] for m in grp}
                    accums = []
                    for wi in range(2):
                        accums.append(ps.tile([128, SCW], F32, name=f"gp{wi}",
                                              tag="proj", bufs=2))
                    for wi in range(2):
                        gsc = ps.tile([128, 2 * SCW], F32, name=f"gsc{wi}",
                                      tag="sc", bufs=2)
                        accums.append(gsc[:, 0:SCW])
                        accums.append(gsc[:, SCW:2 * SCW])
                    for wi in range(2):
                        accums.append(ps.tile([128, SCW], F32, name=f"go{wi}",
                                              tag="o", bufs=2))
                    pp8 = {}
                    for gi, (m, scq) in enumerate(
                            (m, s) for m in grp for s in range(2)):
                        pp8[(m, scq)] = accums[gi]
                    for kc in range(NKC):
                        for m in grp:
                            for scq in range(2):
                                nc.tensor.matmul(
                                    pp8[(m, scq)],
                                    wsl4[m][:, kc * 128:(kc + 1) * 128],
                                    xts[(0, kc)][:, scq * SCW:(scq + 1) * SCW],
                                    start=(kc == 0), stop=(kc == NKC - 1),
                                )
                    # K ropes first, then free ALL V accumulators via ACT
                    # copies before any pv transpose reuses their sc slots
                    for m in (NQL, NQL + 1):
                        for scq in range(2):
                            rope_or_v(0, m, scq, pp8[(m, scq)], qtiles)
                    gvts = {}
                    for m in (NQL + NKVL, NQL + NKVL + 1):
                        for scq in range(2):
                            kv = m - NQL - NKVL
                            vts = sb.tile([128, SCW], BF16,
                                          name=f"gv{kv}_{scq}", tag="vts",
                                          bufs=4)
                            nc.scalar.copy(out=vts[:], in_=pp8[(m, scq)])
                            gvts[(kv, scq)] = vts
                    for (kv, scq), vts in gvts.items():
                        transpose_v(0, kv, scq, vts)
                    morder = list(range(NQL))
                else:
                    # K heads already projected via the hf0-attention
                    # filler; V heads then q heads remain
                    morder = [NQL + NKVL, NQL + NKVL + 1] + list(range(NQL))
                for m in morder:
                    wsl = load_wsl(hf, m)
                    if m < NQL:
                        qt = sb.tile([128, HW], BF16, name=f"q{hf}_{m}",
                                     tag=f"q{m}", bufs=1)
                        qtiles[m] = qt
                    for scq in range(2):
                        qsl = slice(scq * SCW, (scq + 1) * SCW)
                        pp = ps.tile([128, SCW], F32, name=f"pp{hf}_{m}_{scq}",
                                     tag="proj", bufs=2)
                        for kc in range(NKC):
                            nc.tensor.matmul(
                                pp[:], wsl[:, kc * 128:(kc + 1) * 128],
                                xts[(hf, kc)][:, qsl],
                                start=(kc == 0), stop=(kc == NKC - 1),
                            )
                        rope_or_v(hf, m, scq, pp, qtiles)

                if hf == 0:
                    # hf1 K-head weights first so the attention filler's
                    # first matmuls aren't stuck behind the x DMA queue
                    load_wsl(1, NQL)
                    load_wsl(1, NQL + 1)
                    load_xt(1)
                    if DEBUG_DUMP:
                        nc.sync.dma_start(out=dbg["dbg_q0"][:, :], in_=qtiles[0][:])

                    def kv1_gen():
                        # hf1 K-head projections, injected into hf0 attn
                        for m in (NQL, NQL + 1):
                            wsl1 = wsls[(1, m)]
                            for scq in range(2):
                                pp1 = ps.tile([128, SCW], F32,
                                              name=f"kp{m}_{scq}",
                                              tag="proj", bufs=2)
                                for kc in range(NKC):
                                    nc.tensor.matmul(
                                        pp1[:],
                                        wsl1[:, kc * 128:(kc + 1) * 128],
                                        xts[(1, kc)][:, scq * SCW:(scq + 1) * SCW],
                                        start=(kc == 0), stop=(kc == NKC - 1),
                                    )
                                    yield
                                rope_or_v(1, m, scq, pp1, qtiles)

                    kv1 = kv1_gen()
                    attention_half(0, qtiles, kv1, 2.0)
                    for _ in kv1:
                        pass
                else:
                    wo0 = wo_gen(0, 0, S // 256, last_pass=False)
                    attention_half(1, qtiles, wo0, 1.5)

            # drain remaining wo work: rest of first half, then second half
            wo_drain[0] = True
            for _ in wo0:
                pass
            for _ in wo_gen(1, S // 256, S // 128, last_pass=True):
                pass
            if DEBUG_DUMP:
                nc.sync.dma_start(out=dbg["dbg_ksb"][:, :], in_=ksb[:])
                nc.sync.dma_start(out=dbg["dbg_vsb"][:, :], in_=vsb[:])
                nc.sync.dma_start(out=dbg["dbg_at0"][:, :], in_=attnT[0][:])
    nc.finalize()
    return nc


_NC_CACHE = None


def _get_graph():
    global _NC_CACHE
    if _NC_CACHE is None:
        _NC_CACHE = _build()
    return _NC_CACHE


_PERM = np.concatenate([np.arange(0, HD, 2), np.arange(1, HD, 2)])


def _tile_w(w):
    """[D, M*HD] -> [128, m-major kc-major 128cols] contiguous tiling (bf16)."""
    d, mc = w.shape
    nm = mc // HD
    t = w.reshape(NKC, 128, nm, HD).transpose(1, 2, 0, 3)
    return np.ascontiguousarray(t.reshape(128, nm * NKC * HD).astype(NPBF))


def _tile_wo(w):
    """[NQL*HD, D] -> [128, dc-major kc-major 512cols] (bf16)."""
    t = w.reshape(NQL, 128, D // SCW, SCW).transpose(1, 2, 0, 3)
    return np.ascontiguousarray(
        t.reshape(128, (D // SCW) * NQL * SCW).astype(NPBF))


def _shard_inputs(x, freqs_cos, freqs_sin, wq, wk, wv, wo):
    """Build the 8 per-core input maps (pure numpy prep, nothing on-device)."""
    x = np.asarray(x, dtype=np.float32)
    wq = np.asarray(wq, dtype=np.float32)
    wk = np.asarray(wk, dtype=np.float32)
    wv = np.asarray(wv, dtype=np.float32)
    wo = np.asarray(wo, dtype=np.float32)
    cos = np.asarray(freqs_cos, dtype=np.float32)
    sin = np.asarray(freqs_sin, dtype=np.float32)

    # RoPE tables in on-chip [128, S] layout
    cos2 = np.empty((128, S), np.float32)
    sin2n = np.empty((128, S), np.float32)
    cos2[0:64] = cos.T
    cos2[64:128] = cos.T
    sin2n[0:64] = -sin.T
    sin2n[64:128] = sin.T
    cos2 = cos2.astype(NPBF)
    sin2n = sin2n.astype(NPBF)

    wq4 = wq.reshape(D, NH, HD)
    wk4 = wk.reshape(D, NKV, HD)
    wv4 = wv.reshape(D, NKV, HD)
    wo4 = wo.reshape(NH, HD, D)

    # x transposed + bf16 per batch
    xts = [np.ascontiguousarray(x[b].T.astype(NPBF)) for b in range(B)]

    in_maps = []
    for c in range(NCORES):
        b, g = divmod(c, TPG)
        qh = slice(g * NQL, (g + 1) * NQL)
        kvh = slice(g * NKVL, (g + 1) * NKVL)
        m = {
            "xt": xts[b],
            "wq": _tile_w(wq4[:, qh, :][:, :, _PERM].reshape(D, NQL * HD)),
            "wk": _tile_w(wk4[:, kvh, :][:, :, _PERM].reshape(D, NKVL * HD)),
            "wv": _tile_w(wv4[:, kvh, :].reshape(D, NKVL * HD)),
            "wo": _tile_wo(wo4[qh].reshape(NQL * HD, D)),
            "cosp": cos2,
            "sinp": sin2n,
        }
        in_maps.append(m)
    return in_maps


def kernel(x, start_pos, freqs_cos, freqs_sin, mask, wq, wk, wv, wo,
           cache_k, cache_v):
    x = np.asarray(x)
    in_maps = _shard_inputs(x, freqs_cos, freqs_sin, wq, wk, wv, wo)
    nc = _get_graph()
    res = run_bass_kernel_spmd(nc, in_maps, core_ids=list(range(NCORES)))
    out = np.zeros((B, S, D), dtype=np.float32)
    for b in range(B):
        acc = np.asarray(res.results[b * TPG]["out"]).astype(np.float32)
        for g in range(1, TPG):
            acc += np.asarray(res.results[b * TPG + g]["out"]).astype(np.float32)
        out[b] = acc
    return out


# revision 45
# speedup vs baseline: 1.2012x; 1.0012x over previous
"""Distributed Trainium2 Bass kernel for GQA attention prefill.

Problem: B=2, S=2048, D=4096, 32 q heads, 8 kv heads, head_dim=128, RoPE,
causal mask, start_pos=0.

Sharding (8 cores): DP2 over batch x TP4 over heads.  Core c = b*4 + g gets
batch b, q-heads 8g..8g+7, kv-heads 2g..2g+1, wo rows for those q-heads.
Each core computes a partial [S, D] output (bf16); the host sums the 4
partials per batch (the row-parallel wo unshard).

All layout work happens on the host: x arrives pre-transposed and pre-cast
to bf16, weights arrive bf16 pre-tiled, RoPE tables arrive precomputed in
their on-chip [128, S] layout.

On-core dataflow per half (1024 query positions):
  load xT bf16 tiles [128, 1024]; QKV projection (bf16 matmuls); RoPE on
  the projection PSUM.  Attention runs a pair-pipelined software pipeline:
  each PAIR of 128-key score tiles lands in one 2-bank PSUM pool tile, exp
  runs one ACTIVATE per pair (halving ACT instruction overhead), a DVE
  pre-reduction sums each full pair of P tiles so the softmax denominator
  needs one ones-matmul per pair instead of two (cuts PE rowsum cycles),
  and the causal mask is accumulated by an extra PE matmul (ident x mask)
  instead of a DVE add, keeping DVE off the critical path.  PE bubbles in
  the ACT/DVE-paced attention phases are filled with injected matmuls:
  the next half's K-head projections during the first half's attention,
  and the wo output projection for s<1024 during the second half's.  The
  remaining wo work runs as a tail (wo streamed once per s-half), with
  PSUM->SBUF copies on ACT and output DMAs alternating rings.
"""

import math

import numpy as np
import ml_dtypes

import concourse.bass as bass  # noqa: F401  (bass types via bacc)
import concourse.mybir as mybir
from concourse import bacc
from concourse.bass_utils import run_bass_kernel_spmd
from concourse.tile import TileContext

F32 = mybir.dt.float32
BF16 = mybir.dt.bfloat16
NPBF = ml_dtypes.bfloat16

B, S, D = 2, 2048, 4096
NH, NKV, HD = 32, 8, 128
NCORES = 8
TPG = 4                  # tensor-parallel groups
NQL = NH // TPG          # 8 local q heads
NKVL = NKV // TPG        # 2 local kv heads
SCW = 512                # s-chunk width
HW = S // 2              # half width (1024)
NKC = D // 128           # 32 contraction chunks for projections
NTC = S // 128           # 16 T-chunks (key positions)
SCALE = 1.0 / math.sqrt(HD)
NEG = -1e9
MERGE_EXP = True      # one ACTIVATE per adjacent-slot score pair
PREREDUCE = True      # DVE pair pre-reduction + single rowsum matmul
DEBUG_DUMP = False    # add debug DRAM outputs (sim bisection only)


def _build():
    nc = bacc.Bacc("TRN2", target_bir_lowering=False, debug=False,
                   num_devices=NCORES)
    # x pre-transposed + pre-cast: [D, S] bf16
    xt_d = nc.declare_dram_parameter("xt", [D, S], BF16, isOutput=False)
    # weights arrive pre-tiled bf16: [128, m-major kc-major cols]
    wq = nc.declare_dram_parameter("wq", [128, NQL * NKC * HD], BF16, isOutput=False)
    wk = nc.declare_dram_parameter("wk", [128, NKVL * NKC * HD], BF16, isOutput=False)
    wv = nc.declare_dram_parameter("wv", [128, NKVL * NKC * HD], BF16, isOutput=False)
    wo = nc.declare_dram_parameter("wo", [128, (D // SCW) * NQL * SCW], BF16, isOutput=False)
    # RoPE tables in on-chip layout [128, S]
    cosp = nc.declare_dram_parameter("cosp", [128, S], BF16, isOutput=False)
    sinp = nc.declare_dram_parameter("sinp", [128, S], BF16, isOutput=False)
    out = nc.declare_dram_parameter("out", [S, D], BF16, isOutput=True)
    if DEBUG_DUMP:
        dbg = {
            "dbg_ksb": nc.declare_dram_parameter("dbg_ksb", [128, NKVL * S], BF16, isOutput=True),
            "dbg_vsb": nc.declare_dram_parameter("dbg_vsb", [128, NTC * NKVL * HD], BF16, isOutput=True),
            "dbg_q0": nc.declare_dram_parameter("dbg_q0", [128, HW], BF16, isOutput=True),
            "dbg_at0": nc.declare_dram_parameter("dbg_at0", [128, S], BF16, isOutput=True),
            "dbg_rec": nc.declare_dram_parameter("dbg_rec", [128, 4 * SCW], F32, isOutput=True),
            "dbg_po": nc.declare_dram_parameter("dbg_po", [128, 4 * SCW], F32, isOutput=True),
        }

    NM = NQL + 2 * NKVL

    with TileContext(nc) as tc:
        with (
            tc.tile_pool(name="const", bufs=1) as const,
            tc.tile_pool(name="big", bufs=1) as big,
            tc.tile_pool(name="sb", bufs=3) as sb,
            tc.tile_pool(name="ps", bufs=1, space="PSUM") as ps,
        ):
            # ---- weight-slice loader ----
            wsls = {}

            def load_wsl(hf, m, tag="wsl", bufs=4):
                if (hf, m) in wsls:
                    return wsls[(hf, m)]
                wsl = sb.tile([128, NKC * HD], BF16, name=f"w{hf}_{m}",
                              tag=tag, bufs=bufs)
                if m < NQL:
                    base = wq
                    m0 = m
                elif m < NQL + NKVL:
                    base = wk
                    m0 = m - NQL
                else:
                    base = wv
                    m0 = m - NQL - NKVL
                qw = NKC * HD // 4
                for q4 in range(4):
                    c0 = m0 * NKC * HD + q4 * qw
                    # split the weight stream across two rings so the
                    # Q-head loops never catch up with a clogged DMA queue;
                    # hf0 Q-heads use sync (not scalar) so their descriptor
                    # issues don't sit ahead of the attention exps in the
                    # ACT queue, and the pre-attention hf1 KV loads stay on
                    # gpsimd entirely for the same reason (their slot WAR
                    # releases mid-attention and would head-of-line block
                    # the first exps)
                    if q4 % 2 == 0:
                        eng = nc.gpsimd
                    elif hf == 0 and m < NQL:
                        eng = nc.sync
                    elif hf == 1 and m >= NQL:
                        eng = nc.gpsimd
                    else:
                        eng = nc.scalar
                    eng.dma_start(out=wsl[:, q4 * qw:(q4 + 1) * qw],
                                  in_=base[:, c0:c0 + qw])
                wsls[(hf, m)] = wsl
                return wsl

            # ---- xT loads: per (hf, kc) one tile [128, 1024].  The first
            # few kc tiles are issued before everything else so the 8-core
            # HBM rush at startup serves the critical path first ----
            xts = {}

            def load_xt(hf, kcs=None):
                for kc in (range(NKC) if kcs is None else kcs):
                    if (hf, kc) in xts:
                        continue
                    t = sb.tile([128, HW], BF16, name=f"xt{hf}_{kc}",
                                tag="xt", bufs=32)
                    if kc % 2 == 0:
                        eng = nc.sync
                    elif hf == 0:
                        eng = nc.scalar
                    else:
                        eng = nc.gpsimd
                    eng.dma_start(
                        out=t[:],
                        in_=xt_d[kc * 128:(kc + 1) * 128, hf * HW:(hf + 1) * HW])
                    xts[(hf, kc)] = t

            load_xt(0, range(6))

            # K/V head weights for the grouped pass: interleave the four
            # column-quarters across heads so every head's first quarter
            # lands before any head's second quarter
            for m0_ in range(NQL, NQL + 2 * NKVL):
                wsls[(0, m0_)] = sb.tile([128, NKC * HD], BF16,
                                         name=f"w0_{m0_}", tag="wsl", bufs=4)
            qw0 = NKC * HD // 4
            for q4 in range(4):
                for m0_ in range(NQL, NQL + 2 * NKVL):
                    if m0_ < NQL + NKVL:
                        base0 = wk
                        mb0 = m0_ - NQL
                    else:
                        base0 = wv
                        mb0 = m0_ - NQL - NKVL
                    c0 = mb0 * NKC * HD + q4 * qw0
                    eng0 = nc.scalar if q4 % 2 == 1 else nc.gpsimd
                    eng0.dma_start(
                        out=wsls[(0, m0_)][:, q4 * qw0:(q4 + 1) * qw0],
                        in_=base0[:, c0:c0 + qw0])

            # ---- constants ----
            ident = const.tile([128, 128], BF16, name="ident")
            nc.gpsimd.memset(ident[:], 0.0)
            nc.gpsimd.affine_select(
                out=ident[:], in_=ident[:],
                compare_op=mybir.AluOpType.not_equal, fill=1.0,
                base=0, pattern=[[-1, 128]], channel_multiplier=1,
            )
            ones = const.tile([128, 128], BF16, name="ones")
            nc.gpsimd.memset(ones[:], 1.0)
            # causal mask as a bf16 matmul operand: applied to the score
            # PSUM via an accumulating PE matmul (keeps the mask off DVE)
            maskbf = const.tile([128, 128], BF16, name="maskbf")
            nc.gpsimd.memset(maskbf[:], 0.0)
            nc.gpsimd.affine_select(
                out=maskbf[:], in_=maskbf[:],
                compare_op=mybir.AluOpType.is_ge, fill=NEG,
                base=0, pattern=[[1, 128]], channel_multiplier=-1,
            )
            # RoPE tables ride the sync ring (empty at start) so the PE
            # warmup matmuls aren't stuck behind the weight-quarter stream
            cos2 = const.tile([128, S], BF16, name="cos2")
            sin2n = const.tile([128, S], BF16, name="sin2n")
            for hh in range(2):
                hsl = slice(hh * (S // 2), (hh + 1) * (S // 2))
                nc.sync.dma_start(out=cos2[:, hsl], in_=cosp[:, hsl])
                nc.sync.dma_start(out=sin2n[:, hsl], in_=sinp[:, hsl])

            # PE p-state warmup: dummy matmuls ramp the array to full clock
            # (touch every PSUM tag: 2 proj + 2 sc pair-tiles + 2 o)
            warm_dsts = []
            for wi in range(2):
                warm_dsts.append(ps.tile([128, SCW], F32, name=f"pwp{wi}",
                                         tag="proj", bufs=2))
            for wi in range(2):
                wsc = ps.tile([128, 2 * SCW], F32, name=f"pws{wi}",
                              tag="sc", bufs=2)
                warm_dsts.append(wsc[:, 0:SCW])
                warm_dsts.append(wsc[:, SCW:2 * SCW])
            for wi in range(2):
                warm_dsts.append(ps.tile([128, SCW], F32, name=f"pwo{wi}",
                                         tag="o", bufs=2))
            ksb = big.tile([128, NKVL * S], BF16, name="ksb")
            vsb = big.tile([128, NTC * NKVL * HD], BF16, name="vsb")
            # attention outputs for the full sequence
            attnT = [big.tile([128, S], BF16, name=f"at{h}") for h in range(NQL)]

            # warmup copies scribble into attnT (overwritten later)
            for wi, pw in enumerate(warm_dsts):
                nc.tensor.matmul(pw, cos2[:, 0:128], cos2[:, 0:SCW],
                                 start=True, stop=True)
                nc.vector.tensor_copy(
                    out=attnT[wi // 4][:, (wi % 4) * SCW:(wi % 4 + 1) * SCW],
                    in_=pw)

            # remaining xT tiles (first few were front-loaded above)
            load_xt(0)

            # the first two Q-heads' weights ride the (idle-until-wo) wot
            # slots: fresh buffers with no WAR on the grouped pass, so the
            # Q-head loop starts with its weights already resident
            load_wsl(0, 0, tag="wot", bufs=2)
            load_wsl(0, 1, tag="wot", bufs=2)

            def rope_or_v(hf, m, scq, pp, qtiles):
                sc = hf * 2 + scq
                ssl = slice(sc * SCW, (sc + 1) * SCW)
                qsl = slice(scq * SCW, (scq + 1) * SCW)
                if m < NQL + NKVL:
                    if m < NQL:
                        dst = qtiles[m][:, qsl]
                    else:
                        kv = m - NQL
                        dst = ksb[:, kv * S + sc * SCW:kv * S + (sc + 1) * SCW]
                    t1 = sb.tile([128, SCW], BF16, name=f"t1_{hf}_{m}_{scq}",
                                 tag="t1", bufs=2)
                    t2 = sb.tile([128, SCW], BF16, name=f"t2_{hf}_{m}_{scq}",
                                 tag="t2", bufs=2)
                    nc.vector.tensor_tensor(
                        out=t1[0:64, :], in0=pp[64:128, :],
                        in1=sin2n[0:64, ssl], op=mybir.AluOpType.mult)
                    nc.vector.tensor_tensor(
                        out=t1[64:128, :], in0=pp[0:64, :],
                        in1=sin2n[64:128, ssl], op=mybir.AluOpType.mult)
                    nc.vector.tensor_tensor(
                        out=t2[:], in0=pp[:], in1=cos2[:, ssl],
                        op=mybir.AluOpType.mult)
                    nc.vector.tensor_tensor(
                        out=dst, in0=t1[:], in1=t2[:],
                        op=mybir.AluOpType.add)
                else:
                    kv = m - NQL - NKVL
                    vts = sb.tile([128, SCW], BF16, name=f"vts{hf}_{kv}_{scq}",
                                  tag="vts", bufs=4)
                    nc.scalar.copy(out=vts[:], in_=pp)
                    transpose_v(hf, kv, scq, vts)

            def transpose_v(hf, kv, scq, vts):
                sc = hf * 2 + scq
                for j in range(SCW // 128):
                    pv = ps.tile([128, 128], BF16,
                                 name=f"pv{hf}_{kv}_{scq}_{j}",
                                 tag="sc", bufs=2)
                    nc.tensor.transpose(
                        pv[:], vts[:, j * 128:(j + 1) * 128], ident[:])
                    slot = (sc * 4 + j) * NKVL + kv
                    nc.scalar.copy(
                        out=vsb[:, slot * HD:(slot + 1) * HD], in_=pv[:])

            # ---- wo output projection as a resumable generator ----------
            oblk = NQL * SCW
            wots = {}

            def load_wot(li, dc):
                wot = sb.tile([128, NQL * SCW], BF16, name=f"wot{li}_{dc}",
                              tag="wot", bufs=2)
                qw = oblk // 4
                for q4 in range(4):
                    nc.gpsimd.dma_start(
                        out=wot[:, q4 * qw:(q4 + 1) * qw],
                        in_=wo[:, dc * oblk + q4 * qw:dc * oblk + (q4 + 1) * qw])
                wots[(li, dc)] = wot

            # after the attention phases complete, the sc/o PSUM banks are
            # idle: drain-mode wo accumulators cycle through all three tags
            # (6-deep pipeline) so dc-boundary os-copy chains never stall PE
            wo_drain = [False]

            def wo_gen(li, ssub_lo, ssub_hi, last_pass):
                """Yields once per wo matmul; boundary ops emitted inline."""
                load_wot(li, 0)
                load_wot(li, 1)
                cnt = 0
                for dc in range(D // SCW):
                    wot = wots[(li, dc)]
                    if dc + 2 < D // SCW:
                        load_wot(li, dc + 2)
                    for ssub in range(ssub_lo, ssub_hi):
                        s0 = ssub * 128
                        tg = ("proj", "sc", "o")[cnt % 3] if wo_drain[0] else "proj"
                        pd = ps.tile([128, SCW], F32, name=f"pd{li}_{dc}_{ssub}",
                                     tag=tg, bufs=2)
                        for kc8 in range(NQL):
                            nc.tensor.matmul(
                                pd[:],
                                attnT[kc8][:, s0:s0 + 128],
                                wot[:, kc8 * SCW:(kc8 + 1) * SCW],
                                start=(kc8 == 0), stop=(kc8 == NQL - 1))
                            yield
                        os_ = sb.tile([128, SCW], BF16, name=f"os{li}_{dc}_{ssub}",
                                      tag="os", bufs=4)
                        if wo_drain[0] and cnt % 2 == 0:
                            nc.scalar.copy(out=os_[:], in_=pd[:])
                        else:
                            nc.vector.tensor_copy(out=os_[:], in_=pd[:])
                        if last_pass and dc == D // SCW - 1:
                            oeng = (nc.sync, nc.gpsimd, nc.scalar)[ssub % 3]
                        else:
                            oeng = nc.sync if ssub % 2 == 0 else nc.gpsimd
                        oeng.dma_start(
                            out=out[s0:s0 + 128, dc * SCW:(dc + 1) * SCW],
                            in_=os_[:])
                        cnt += 1

            # ---- attention for one half: pair-pipelined ------------------
            def attention_half(hf, qtiles, filler, fill_rate):
                pairs = []
                for h in range(NQL):
                    for scq in range(2):
                        sc = hf * 2 + scq
                        for k in range(2 * sc + 2):
                            pairs.append((h, scq, k))
                n = len(pairs)
                psps = {}    # (h, scq, k) -> score pair tile [128, 1024] PSUM
                pts = {}     # (h, scq, k) -> pt pair tile [128, 1024]
                tmps = {}    # (h, scq, k) -> prereduced tile (full pairs)
                pos = {}
                prs = {}
                fill_acc = [0.0]

                def geom(sc, tcx):
                    j = tcx - 4 * sc
                    off = j * 128 if j > 0 else 0
                    return j, off, SCW - off

                def stage_a(h, scq, k):
                    # scores for both tiles of the pair + diagonal masks
                    sc = hf * 2 + scq
                    kv = h // (NQL // NKVL)
                    psp = ps.tile([128, 2 * SCW], F32, name=f"psp{sc}_{h}_{k}",
                                  tag="sc", bufs=2)
                    psps[(h, scq, k)] = psp
                    for half in range(2):
                        tcx = 2 * k + half
                        j, off, w = geom(sc, tcx)
                        qs0 = scq * SCW + off
                        diag = j >= 0
                        nc.tensor.matmul(
                            psp[:, half * SCW:half * SCW + w],
                            ksb[:, kv * S + tcx * 128:kv * S + (tcx + 1) * 128],
                            qtiles[h][:, qs0:qs0 + w],
                            start=True, stop=not diag,
                        )
                        if diag:
                            # accumulate the causal mask on PE (keeps DVE
                            # out of the attention critical path)
                            nc.tensor.matmul(
                                psp[:, half * SCW:half * SCW + 128],
                                ident[:], maskbf[:],
                                start=False, stop=True)

                def stage_b(h, scq, k):
                    # one exp ACTIVATE per pair, then DVE pre-reduction
                    sc = hf * 2 + scq
                    psp = psps.pop((h, scq, k))
                    _, _, w0 = geom(sc, 2 * k)
                    _, _, w1 = geom(sc, 2 * k + 1)
                    pt = sb.tile([128, 2 * SCW], BF16, name=f"pt{sc}_{h}_{k}",
                                 tag="pt", bufs=2)
                    if MERGE_EXP and w0 == SCW:
                        # contiguous 2D span over both halves (only when the
                        # first half is fully written - no junk-gap reads)
                        nc.scalar.activation(
                            pt[:, 0:SCW + w1], psp[:, 0:SCW + w1],
                            mybir.ActivationFunctionType.Exp, scale=SCALE)
                    else:
                        nc.scalar.activation(
                            pt[:, 0:w0], psp[:, 0:w0],
                            mybir.ActivationFunctionType.Exp, scale=SCALE)
                        nc.scalar.activation(
                            pt[:, SCW:SCW + w1], psp[:, SCW:SCW + w1],
                            mybir.ActivationFunctionType.Exp, scale=SCALE)
                    pts[(h, scq, k)] = pt
                    if PREREDUCE and w0 == SCW and w1 == SCW:
                        tmp = sb.tile([128, SCW], BF16, name=f"tm{sc}_{h}_{k}",
                                      tag="tmp", bufs=2)
                        nc.vector.tensor_tensor(
                            out=tmp[:], in0=pt[:, 0:SCW], in1=pt[:, SCW:2 * SCW],
                            op=mybir.AluOpType.add)
                        tmps[(h, scq, k)] = tmp

                def stage_c(h, scq, k):
                    # PV matmuls for both tiles of the pair
                    sc = hf * 2 + scq
                    ntc = 4 * sc + 4
                    kv = h // (NQL // NKVL)
                    if k == 0:
                        # po and pr alternate through the same 2-buf tag
                        pos[(h, scq)] = ps.tile([128, SCW], F32,
                                                name=f"po{sc}_{h}", tag="o",
                                                bufs=2)
                        prs[(h, scq)] = ps.tile([128, SCW], F32,
                                                name=f"pr{sc}_{h}", tag="o",
                                                bufs=2)
                    po = pos[(h, scq)]
                    pt = pts[(h, scq, k)]
                    for half in range(2):
                        tcx = 2 * k + half
                        j, off, w = geom(sc, tcx)
                        slot = tcx * NKVL + kv
                        nc.tensor.matmul(
                            po[:, off:], vsb[:, slot * HD:(slot + 1) * HD],
                            pt[:, half * SCW:half * SCW + w],
                            start=(tcx == 0), stop=(tcx == ntc - 1))

                def stage_d(h, scq, k):
                    # rowsum matmul(s) for the pair; normalize at group end
                    sc = hf * 2 + scq
                    npr = 2 * sc + 2
                    pr = prs[(h, scq)]
                    pt = pts.pop((h, scq, k))
                    tmp = tmps.pop((h, scq, k), None)
                    if tmp is not None:
                        nc.tensor.matmul(
                            pr[:], ones[:], tmp[:],
                            start=(k == 0), stop=False)
                    else:
                        for half in range(2):
                            tcx = 2 * k + half
                            j, off, w = geom(sc, tcx)
                            nc.tensor.matmul(
                                pr[:, off:], ones[:], pt[:, half * SCW:half * SCW + w],
                                start=(k == 0 and half == 0),
                                stop=(k == npr - 1 and half == 1))
                    if k == npr - 1:
                        po = pos.pop((h, scq))
                        prs.pop((h, scq))
                        rec = sb.tile([128, SCW], F32, name=f"rec{sc}_{h}",
                                      tag="rec", bufs=1)
                        nc.vector.reciprocal_approx_fast(out=rec[:], in_=pr[:])
                        if DEBUG_DUMP and h == 0:
                            pcp = sb.tile([128, SCW], F32, name=f"pcp{sc}",
                                          tag="dbgp", bufs=1)
                            nc.vector.tensor_copy(out=pcp[:], in_=po[:])
                            nc.sync.dma_start(
                                out=dbg["dbg_po"][:, sc * SCW:(sc + 1) * SCW],
                                in_=pcp[:])
                            nc.sync.dma_start(
                                out=dbg["dbg_rec"][:, sc * SCW:(sc + 1) * SCW],
                                in_=rec[:])
                        nc.vector.tensor_tensor(
                            out=attnT[h][:, sc * SCW:(sc + 1) * SCW],
                            in0=po[:], in1=rec[:],
                            op=mybir.AluOpType.mult)

                for i in range(n + 2):
                    if i < n:
                        stage_a(*pairs[i])
                    if 0 <= i - 1 < n:
                        stage_b(*pairs[i - 1])
                    if filler is not None and i >= 2:
                        fill_acc[0] += fill_rate
                        while fill_acc[0] >= 1.0:
                            if next(filler, None) is None:
                                filler = None
                                break
                            fill_acc[0] -= 1.0
                    if 0 <= i - 2 < n:
                        stage_c(*pairs[i - 2])
                        stage_d(*pairs[i - 2])

            # ================= main schedule =================
            wo0 = None
            for hf in range(2):
                qtiles = [None] * NQL
                if hf == 0:
                    # grouped pass over the 4 K/V heads x both s-chunks
                    grp = list(range(NQL, NM))
                    wsl4 = {m: wsls[(0, m)] for m in grp}
                    accums = []
                    for wi in range(2):
                        accums.append(ps.tile([128, SCW], F32, name=f"gp{wi}",
                                              tag="proj", bufs=2))
                    for wi in range(2):
                        gsc = ps.tile([128, 2 * SCW], F32, name=f"gsc{wi}",
                                      tag="sc", bufs=2)
                        accums.append(gsc[:, 0:SCW])
                        accums.append(gsc[:, SCW:2 * SCW])
                    for wi in range(2):
                        accums.append(ps.tile([128, SCW], F32, name=f"go{wi}",
                                              tag="o", bufs=2))
                    pp8 = {}
                    for gi, (m, scq) in enumerate(
                            (m, s) for m in grp for s in range(2)):
                        pp8[(m, scq)] = accums[gi]
                    for kc in range(NKC):
                        for m in grp:
                            for scq in range(2):
                                nc.tensor.matmul(
                                    pp8[(m, scq)],
                                    wsl4[m][:, kc * 128:(kc + 1) * 128],
                                    xts[(0, kc)][:, scq * SCW:(scq + 1) * SCW],
                                    start=(kc == 0), stop=(kc == NKC - 1),
                                )
                    # K ropes first, then free ALL V accumulators via ACT
                    # copies before any pv transpose reuses their sc slots
                    for m in (NQL, NQL + 1):
                        for scq in range(2):
                            rope_or_v(0, m, scq, pp8[(m, scq)], qtiles)
                    gvts = {}
                    for m in (NQL + NKVL, NQL + NKVL + 1):
                        for scq in range(2):
                            kv = m - NQL - NKVL
                            vts = sb.tile([128, SCW], BF16,
                                          name=f"gv{kv}_{scq}", tag="vts",
                                          bufs=4)
                            nc.scalar.copy(out=vts[:], in_=pp8[(m, scq)])
                            gvts[(kv, scq)] = vts
                    for (kv, scq), vts in gvts.items():
                        transpose_v(0, kv, scq, vts)
                    morder = list(range(NQL))
                else:
                    # K heads already projected via the hf0-attention
                    # filler; V heads then q heads remain
                    morder = [NQL + NKVL, NQL + NKVL + 1] + list(range(NQL))
                for m in morder:
                    wsl = load_wsl(hf, m)
                    if m < NQL:
                        qt = sb.tile([128, HW], BF16, name=f"q{hf}_{m}",
                                     tag=f"q{m}", bufs=1)
                        qtiles[m] = qt
                    for scq in range(2):
                        qsl = slice(scq * SCW, (scq + 1) * SCW)
                        pp = ps.tile([128, SCW], F32, name=f"pp{hf}_{m}_{scq}",
                                     tag="proj", bufs=2)
                        for kc in range(NKC):
                            nc.tensor.matmul(
                                pp[:], wsl[:, kc * 128:(kc + 1) * 128],
                                xts[(hf, kc)][:, qsl],
                                start=(kc == 0), stop=(kc == NKC - 1),
                            )
                        rope_or_v(hf, m, scq, pp, qtiles)

                if hf == 0:
                    # hf1 K-head weights first so the attention filler's
                    # first matmuls aren't stuck behind the x DMA queue
                    load_wsl(1, NQL)
                    load_wsl(1, NQL + 1)
                    load_xt(1)
                    if DEBUG_DUMP:
                        nc.sync.dma_start(out=dbg["dbg_q0"][:, :], in_=qtiles[0][:])

                    def kv1_gen():
                        # hf1 K-head projections, injected into hf0 attn
                        for m in (NQL, NQL + 1):
                            wsl1 = wsls[(1, m)]
                            for scq in range(2):
                                pp1 = ps.tile([128, SCW], F32,
                                              name=f"kp{m}_{scq}",
                                              tag="proj", bufs=2)
                                for kc in range(NKC):
                                    nc.tensor.matmul(
                                        pp1[:],
                                        wsl1[:, kc * 128:(kc + 1) * 128],
                                        xts[(1, kc)][:, scq * SCW:(scq + 1) * SCW],
                                        start=(kc == 0), stop=(kc == NKC - 1),
                                    )
                                    yield
                                rope_or_v(1, m, scq, pp1, qtiles)

                    kv1 = kv1_gen()
                    attention_half(0, qtiles, kv1, 2.0)
                    for _ in kv1:
                        pass
                else:
                    wo0 = wo_gen(0, 0, S // 256, last_pass=False)
                    attention_half(1, qtiles, wo0, 1.5)

            # drain remaining wo work: rest of first half, then second half
            wo_drain[0] = True
            for _ in wo0:
                pass
            for _ in wo_gen(1, S // 256, S // 128, last_pass=True):
                pass
            if DEBUG_DUMP:
                nc.sync.dma_start(out=dbg["dbg_ksb"][:, :], in_=ksb[:])
                nc.sync.dma_start(out=dbg["dbg_vsb"][:, :], in_=vsb[:])
                nc.sync.dma_start(out=dbg["dbg_at0"][:, :], in_=attnT[0][:])
    nc.finalize()
    return nc


_NC_CACHE = None


def _get_graph():
    global _NC_CACHE
    if _NC_CACHE is None:
        _NC_CACHE = _build()
    return _NC_CACHE


_PERM = np.concatenate([np.arange(0, HD, 2), np.arange(1, HD, 2)])


def _tile_w(w):
    """[D, M*HD] -> [128, m-major kc-major 128cols] contiguous tiling (bf16)."""
    d, mc = w.shape
    nm = mc // HD
    t = w.reshape(NKC, 128, nm, HD).transpose(1, 2, 0, 3)
    return np.ascontiguousarray(t.reshape(128, nm * NKC * HD).astype(NPBF))


def _tile_wo(w):
    """[NQL*HD, D] -> [128, dc-major kc-major 512cols] (bf16)."""
    t = w.reshape(NQL, 128, D // SCW, SCW).transpose(1, 2, 0, 3)
    return np.ascontiguousarray(
        t.reshape(128, (D // SCW) * NQL * SCW).astype(NPBF))


def _shard_inputs(x, freqs_cos, freqs_sin, wq, wk, wv, wo):
    """Build the 8 per-core input maps (pure numpy prep, nothing on-device)."""
    x = np.asarray(x, dtype=np.float32)
    wq = np.asarray(wq, dtype=np.float32)
    wk = np.asarray(wk, dtype=np.float32)
    wv = np.asarray(wv, dtype=np.float32)
    wo = np.asarray(wo, dtype=np.float32)
    cos = np.asarray(freqs_cos, dtype=np.float32)
    sin = np.asarray(freqs_sin, dtype=np.float32)

    # RoPE tables in on-chip [128, S] layout
    cos2 = np.empty((128, S), np.float32)
    sin2n = np.empty((128, S), np.float32)
    cos2[0:64] = cos.T
    cos2[64:128] = cos.T
    sin2n[0:64] = -sin.T
    sin2n[64:128] = sin.T
    cos2 = cos2.astype(NPBF)
    sin2n = sin2n.astype(NPBF)

    wq4 = wq.reshape(D, NH, HD)
    wk4 = wk.reshape(D, NKV, HD)
    wv4 = wv.reshape(D, NKV, HD)
    wo4 = wo.reshape(NH, HD, D)

    # x transposed + bf16 per batch
    xts = [np.ascontiguousarray(x[b].T.astype(NPBF)) for b in range(B)]

    in_maps = []
    for c in range(NCORES):
        b, g = divmod(c, TPG)
        qh = slice(g * NQL, (g + 1) * NQL)
        kvh = slice(g * NKVL, (g + 1) * NKVL)
        m = {
            "xt": xts[b],
            "wq": _tile_w(wq4[:, qh, :][:, :, _PERM].reshape(D, NQL * HD)),
            "wk": _tile_w(wk4[:, kvh, :][:, :, _PERM].reshape(D, NKVL * HD)),
            "wv": _tile_w(wv4[:, kvh, :].reshape(D, NKVL * HD)),
            "wo": _tile_wo(wo4[qh].reshape(NQL * HD, D)),
            "cosp": cos2,
            "sinp": sin2n,
        }
        in_maps.append(m)
    return in_maps


def kernel(x, start_pos, freqs_cos, freqs_sin, mask, wq, wk, wv, wo,
           cache_k, cache_v):
    x = np.asarray(x)
    in_maps = _shard_inputs(x, freqs_cos, freqs_sin, wq, wk, wv, wo)
    nc = _get_graph()
    res = run_bass_kernel_spmd(nc, in_maps, core_ids=list(range(NCORES)))
    out = np.zeros((B, S, D), dtype=np.float32)
    for b in range(B):
        acc = np.asarray(res.results[b * TPG]["out"]).astype(np.float32)
        for g in range(1, TPG):
            acc += np.asarray(res.results[b * TPG + g]["out"]).astype(np.float32)
        out[b] = acc
    return out
